# revision 7
# baseline (speedup 1.0000x reference)
"""Trainium2 Bass kernel for nn_PointEstimator (segment_reduce).

Computation (per batch row b):
    xc = x - x[:, -1:]
    logits = xc @ W_pts.T + b_pts            # (B, 8*336)
    pts    = sort(argmax over each 336-group) # (B, 8)
    params = (xc @ W_comp.T + b_comp)        # (B, 9, 2) slope/intercept
    out[t] = a[s(t)] * t + c[s(t)] + x[:,-1],  s(t) = #(pts <= t)

Device algorithm highlights:
  * batch rows on PSUM partitions; seq (512) is the contraction dim.
  * matmul in a bf16 3-term split (xh@wh + xh@wl + xl@wh) for fp32-level
    logit accuracy (needed: min top-2 logit gap is ~6e-6); bias folded in
    as a K=2 all-ones matmul that opens each PSUM accumulation group.
  * x is transposed on-device via PE transpose; xc=x-lastcol on DVE.
  * argmax without sort: running-max prefix scan (tensor_tensor_scan max)
    then t* = sum_t [rm_t < rm_335] via scalar_tensor_tensor accum (exact
    first-index tie-break).
  * piecewise combine without sort or gather: for each point, its slot
    jump da = a[cnt_le]-a[cnt_lt] (one-hot lookups via broadcast compare),
    deduplicated to the first point of each tied value, scattered with
    gpsimd local_scatter, then out = double prefix-sum:
        u = a0 + cumsum(j1);  res = (c0+lv-a0) + cumsum(u + j2),
        j1[p] = da, j2[p] = da*(p-1)+dc.
"""

import os
import numpy as np
import ml_dtypes

N_CORES = 8
B_FULL = 8192
SEQ = 512
PRED = 336
NPTS = 8
NSLOTS = 9
NOUT = NPTS * PRED + NSLOTS * 2  # 2688 + 18 = 2706
PARAM_OFF = NPTS * PRED          # 2688

BF = ml_dtypes.bfloat16

_CACHE = {}


def _build(rows):
    """Build + compile the per-core Bass program for `rows` batch rows."""
    import concourse.bass as bass
    import concourse.tile as tile
    from concourse import bacc, mybir
    from concourse import library_config
    from contextlib import ExitStack

    f32 = mybir.dt.float32
    bf16 = mybir.dt.bfloat16
    i16 = mybir.dt.int16
    Alu = mybir.AluOpType

    MT = rows // 128
    assert rows % 128 == 0

    nc = bacc.Bacc("TRN2", target_bir_lowering=False, debug=False)

    x_d = nc.dram_tensor("x", [rows, SEQ], f32, kind="ExternalInput").ap()
    wh_d = nc.dram_tensor("wh", [SEQ, NOUT], bf16, kind="ExternalInput").ap()
    wl_d = nc.dram_tensor("wl", [SEQ, NOUT], bf16, kind="ExternalInput").ap()
    ident_d = nc.dram_tensor("ident", [128, 128], f32, kind="ExternalInput").ap()
    iota9_d = nc.dram_tensor("iota9", [128, NSLOTS], f32, kind="ExternalInput").ap()
    tri_d = nc.dram_tensor("tri", [128, 64], f32, kind="ExternalInput").ap()
    ones_r_d = nc.dram_tensor("ones_r", [1, rows], bf16, kind="ExternalInput").ap()
    zeros_r_d = nc.dram_tensor("zeros_r", [1, rows], bf16, kind="ExternalInput").ap()
    out_d = nc.dram_tensor("out", [rows, PRED], f32, kind="ExternalOutput").ap()

    # Logits live in 3 PSUM tiles of 3 groups / 3 groups / 2 groups+params.
    TSPEC = [
        (0, 1008),        # groups 0..2
        (1008, 2016),     # groups 3..5
        (2016, 2706),     # groups 6..7 + 18 params
    ]

    with tile.TileContext(nc) as tc, ExitStack() as ctx:
        const = ctx.enter_context(tc.tile_pool(name="const", bufs=1))
        wpool = ctx.enter_context(tc.tile_pool(name="w", bufs=1))
        xpool = ctx.enter_context(tc.tile_pool(name="xin", bufs=3))
        xs = ctx.enter_context(tc.tile_pool(name="xsplit", bufs=1))
        work = ctx.enter_context(tc.tile_pool(name="work", bufs=2))
        tiny = ctx.enter_context(tc.tile_pool(name="tiny", bufs=2))
        psum = ctx.enter_context(tc.tile_pool(name="psum", bufs=3, space="PSUM"))
        psumT = ctx.enter_context(tc.tile_pool(name="psumT", bufs=2, space="PSUM"))

        nc.gpsimd.load_library(library_config.local_scatter)

        # ---- constants ----
        ident = const.tile([128, 128], f32)
        nc.sync.dma_start(ident[:], ident_d)
        iota9 = const.tile([128, NSLOTS], f32)
        nc.sync.dma_start(iota9[:], iota9_d)
        tri = const.tile([128, 64], f32)
        nc.sync.dma_start(tri[:], tri_d)
        zeros336 = const.tile([128, PRED], f32)
        nc.vector.memset(zeros336[:], 0.0)

        # ---- weights (resident) ----
        whs, wls = [], []
        for kt in range(4):
            t = wpool.tile([128, NOUT], bf16, tag=f"wh{kt}", name=f"wh{kt}")
            nc.sync.dma_start(t[:], wh_d[kt * 128:(kt + 1) * 128, :])
            whs.append(t)
            t = wpool.tile([128, NOUT], bf16, tag=f"wl{kt}", name=f"wl{kt}")
            nc.sync.dma_start(t[:], wl_d[kt * 128:(kt + 1) * 128, :])
            wls.append(t)

        # ---- persistent transposed x splits ----
        xh = [xs.tile([128, rows], bf16, tag=f"xh{kt}", name=f"xh{kt}") for kt in range(4)]
        xl = [xs.tile([128, rows], bf16, tag=f"xl{kt}", name=f"xl{kt}") for kt in range(4)]
        lvs = xs.tile([128, MT], f32, tag="lvs")  # last value per mtile
        # bias rows: xh[3] partition 127 = 1.0 (multiplies W row 511 = bias),
        # xl[3] partition 127 = 0. Written once; per-mtile copies skip row 127.
        nc.sync.dma_start(xh[3][127:128, :], ones_r_d)
        nc.sync.dma_start(xl[3][127:128, :], zeros_r_d)

        for m in range(MT):
            ms = slice(m * 128, (m + 1) * 128)

            # ================= stage A: load, xc, transpose, split ======
            xt = xpool.tile([128, SEQ], f32, tag="xt")
            nc.sync.dma_start(xt[:], x_d[ms, :])
            xc = xpool.tile([128, SEQ], f32, tag="xc")
            nc.vector.tensor_scalar(
                out=xc[:], in0=xt[:], scalar1=xt[:, 511:512], scalar2=None,
                op0=Alu.subtract)
            nc.scalar.copy(out=lvs[:, m:m + 1], in_=xt[:, 511:512])

            pst = psumT.tile([128, SEQ], f32, tag="ptrans")
            for kt in range(4):
                ks = slice(kt * 128, (kt + 1) * 128)
                nc.tensor.transpose(pst[:, ks], xc[:, ks], ident[:])
            for kt in range(4):
                ks = slice(kt * 128, (kt + 1) * 128)
                np_ = 127 if kt == 3 else 128
                pp = slice(0, np_)
                nc.scalar.copy(out=xh[kt][pp, ms], in_=pst[pp, ks])
                nc.vector.tensor_tensor(
                    out=xl[kt][pp, ms], in0=pst[pp, ks], in1=xh[kt][pp, ms],
                    op=Alu.subtract)

            # ================= stage B: matmuls =========================
            pts_tiles = []
            for (c0, c1) in TSPEC:
                pt = psum.tile([128, c1 - c0], f32, tag="lg", name=f"lg{c0}", padded_shape=[128, 1024])
                pts_tiles.append(pt)
                w = c1 - c0
                blocks = [(0, min(512, w))]
                if w > 512:
                    blocks.append((512, w))
                terms = [(xh, whs), (xh, wls), (xl, whs)]
                for kt in range(4):
                    for ti, (xop, wop) in enumerate(terms):
                        first = (kt == 0 and ti == 0)
                        last = (kt == 3 and ti == 2)
                        for (b0, b1) in blocks:
                            nc.tensor.matmul(
                                pt[:, b0:b1], xop[kt][:, ms],
                                wop[kt][:, c0 + b0:c0 + b1],
                                start=first, stop=last)

            def lg(col0, col1):
                """AP view of logits columns [col0, col1) across T tiles."""
                for (c0, c1), pt in zip(TSPEC, pts_tiles):
                    if col0 >= c0 and col1 <= c1:
                        return pt[:, col0 - c0:col1 - c0]
                raise AssertionError((col0, col1))

            # params to SBUF (strided copy PSUM->SBUF)
            a_sb = tiny.tile([128, NSLOTS], f32, tag="a_sb")
            c_sb = tiny.tile([128, NSLOTS], f32, tag="c_sb")
            pview = lg(PARAM_OFF, NOUT).rearrange("p (k two) -> p k two", two=2)
            nc.scalar.copy(out=a_sb[:], in_=pview[:, :, 0:1].squeeze(2))
            nc.scalar.copy(out=c_sb[:], in_=pview[:, :, 1:2].squeeze(2))

            # ================= stage C: argmax ==========================
            rm = work.tile([128, NPTS * PRED], f32, tag="rm")
            pts = tiny.tile([128, NPTS], f32, tag="pts")
            for g in range(NPTS):
                gs = slice(g * PRED, (g + 1) * PRED)
                nc.vector.tensor_tensor_scan(
                    out=rm[:, gs], data0=lg(g * PRED, (g + 1) * PRED),
                    data1=zeros336[:], initial=-3.0e38,
                    op0=Alu.max, op1=Alu.bypass)
            cnt_scr = work.tile([128, PRED], bf16, tag="cntscr")
            for g in range(NPTS):
                gs = slice(g * PRED, (g + 1) * PRED)
                # t* = sum_t sign(m - rm_t)  (1 while rm<m, 0 from argmax on)
                nc.scalar.activation(
                    out=cnt_scr[:], in_=rm[:, gs],
                    func=mybir.ActivationFunctionType.Sign,
                    bias=rm[:, g * PRED + 335:g * PRED + 336], scale=-1.0,
                    accum_out=pts[:, g:g + 1])

            # ================= stage D: combine =========================
            p_ap = pts[:]
            p_i = p_ap.unsqueeze(2).broadcast_to([128, NPTS, NPTS])
            p_j = p_ap.unsqueeze(1).broadcast_to([128, NPTS, NPTS])
            LT = tiny.tile([128, 64], f32, tag="LT")
            LE = tiny.tile([128, 64], f32, tag="LE")
            nc.vector.tensor_tensor(
                out=LT[:].rearrange("p (i j) -> p i j", j=8), in0=p_i, in1=p_j,
                op=Alu.is_gt)   # LT[i,j] = p_j < p_i
            nc.vector.tensor_tensor(
                out=LE[:].rearrange("p (i j) -> p i j", j=8), in0=p_i, in1=p_j,
                op=Alu.is_ge)   # LE[i,j] = p_j <= p_i
            cnt_lt = tiny.tile([128, NPTS], f32, tag="cnt_lt")
            cnt_le = tiny.tile([128, NPTS], f32, tag="cnt_le")
            nc.vector.tensor_reduce(
                out=cnt_lt[:], in_=LT[:].rearrange("p (i j) -> p i j", j=8),
                axis=mybir.AxisListType.X, op=Alu.add)
            nc.vector.tensor_reduce(
                out=cnt_le[:], in_=LE[:].rearrange("p (i j) -> p i j", j=8),
                axis=mybir.AxisListType.X, op=Alu.add)
            EQ = tiny.tile([128, 64], f32, tag="EQ")
            nc.vector.tensor_tensor(out=EQ[:], in0=LE[:], in1=LT[:],
                                    op=Alu.subtract)
            EQt = tiny.tile([128, 64], f32, tag="EQt")
            nc.vector.tensor_tensor(out=EQt[:], in0=EQ[:], in1=tri[:],
                                    op=Alu.mult)
            Ecnt = tiny.tile([128, NPTS], f32, tag="Ecnt")
            nc.vector.tensor_reduce(
                out=Ecnt[:], in_=EQt[:].rearrange("p (i j) -> p i j", j=8),
                axis=mybir.AxisListType.X, op=Alu.add)
            isf = tiny.tile([128, NPTS], f32, tag="isf")
            nc.vector.tensor_scalar(out=isf[:], in0=Ecnt[:], scalar1=0.0,
                                    scalar2=None, op0=Alu.is_equal)

            # one-hot lookups a[cnt], c[cnt]
            i9 = iota9[:].unsqueeze(1).broadcast_to([128, NPTS, NSLOTS])
            a_b = a_sb[:].unsqueeze(1).broadcast_to([128, NPTS, NSLOTS])
            c_b = c_sb[:].unsqueeze(1).broadcast_to([128, NPTS, NSLOTS])

            def onehot_lookup(cnt, tag):
                eq = tiny.tile([128, NPTS * NSLOTS], f32, tag=f"eq{tag}", name=f"eq{tag}")
                eqv = eq[:].rearrange("p (i k) -> p i k", k=NSLOTS)
                nc.vector.tensor_tensor(
                    out=eqv, in0=cnt[:].unsqueeze(2).broadcast_to(
                        [128, NPTS, NSLOTS]), in1=i9, op=Alu.is_equal)
                outs = []
                for nm, tbl in (("a", a_b), ("c", c_b)):
                    prod = tiny.tile([128, NPTS * NSLOTS], f32,
                                     tag=f"pr{nm}{tag}", name=f"pr{nm}{tag}")
                    pv = prod[:].rearrange("p (i k) -> p i k", k=NSLOTS)
                    nc.vector.tensor_tensor(out=pv, in0=eqv, in1=tbl,
                                            op=Alu.mult)
                    red = tiny.tile([128, NPTS], f32, tag=f"rd{nm}{tag}", name=f"rd{nm}{tag}")
                    nc.vector.tensor_reduce(out=red[:], in_=pv,
                                            axis=mybir.AxisListType.X,
                                            op=Alu.add)
                    outs.append(red)
                return outs

            a_le, c_le = onehot_lookup(cnt_le, "le")
            a_lt, c_lt = onehot_lookup(cnt_lt, "lt")

            da = tiny.tile([128, NPTS], f32, tag="da")
            dc = tiny.tile([128, NPTS], f32, tag="dc")
            nc.vector.tensor_tensor(out=da[:], in0=a_le[:], in1=a_lt[:],
                                    op=Alu.subtract)
            nc.vector.tensor_tensor(out=dc[:], in0=c_le[:], in1=c_lt[:],
                                    op=Alu.subtract)
            # j2 = da*(p-1) + dc
            j2a = tiny.tile([128, NPTS], f32, tag="j2a")
            nc.vector.scalar_tensor_tensor(
                out=j2a[:], in0=pts[:], scalar=1.0, in1=da[:],
                op0=Alu.subtract, op1=Alu.mult)
            j2v = tiny.tile([128, NPTS], f32, tag="j2v")
            nc.vector.tensor_tensor(out=j2v[:], in0=j2a[:], in1=dc[:],
                                    op=Alu.add)
            # scatter index: is_first ? p : -1
            p1f = tiny.tile([128, NPTS], f32, tag="p1f")
            nc.vector.scalar_tensor_tensor(
                out=p1f[:], in0=pts[:], scalar=1.0, in1=isf[:],
                op0=Alu.add, op1=Alu.mult)
            sidxf = tiny.tile([128, NPTS], f32, tag="sidxf")
            nc.vector.tensor_scalar(out=sidxf[:], in0=p1f[:], scalar1=1.0,
                                    scalar2=None, op0=Alu.subtract)
            sidx = tiny.tile([128, NPTS], i16, tag="sidx")
            nc.vector.tensor_copy(out=sidx[:], in_=sidxf[:])

            # bf16 hi/lo split of scatter payloads
            def bsplit(src, tag):
                hi = tiny.tile([128, NPTS], bf16, tag=f"{tag}h", name=f"{tag}h")
                nc.vector.tensor_copy(out=hi[:], in_=src[:])
                lo = tiny.tile([128, NPTS], bf16, tag=f"{tag}l", name=f"{tag}l")
                nc.vector.tensor_tensor(out=lo[:], in0=src[:], in1=hi[:],
                                        op=Alu.subtract)
                return hi, lo

            j1h, j1l = bsplit(da, "j1")
            j2h, j2l = bsplit(j2v, "j2")

            scat = []
            for nm, payload in (("s1h", j1h), ("s1l", j1l),
                                ("s2h", j2h), ("s2l", j2l)):
                s = work.tile([128, PRED], bf16, tag=nm, name=nm)
                nc.gpsimd.local_scatter(
                    out_ap=s[:], data_ap=payload[:], idxs_ap=sidx[:],
                    channels=128, num_elems=PRED, num_idxs=NPTS)
                scat.append(s)

            u = work.tile([128, PRED], f32, tag="u")
            nc.vector.tensor_tensor_scan(
                out=u[:], data0=scat[0][:], data1=scat[1][:],
                initial=a_sb[:, 0:1], op0=Alu.add, op1=Alu.add)
            j2s = work.tile([128, PRED], f32, tag="j2s")
            nc.vector.tensor_tensor(out=j2s[:], in0=scat[2][:],
                                    in1=scat[3][:], op=Alu.add)
            init2 = tiny.tile([128, 1], f32, tag="init2")
            nc.vector.scalar_tensor_tensor(
                out=init2[:], in0=c_sb[:, 0:1], scalar=a_sb[:, 0:1],
                in1=lvs[:, m:m + 1], op0=Alu.subtract, op1=Alu.add)
            res = work.tile([128, PRED], f32, tag="res")
            nc.vector.tensor_tensor_scan(
                out=res[:], data0=u[:], data1=j2s[:],
                initial=init2[:], op0=Alu.add, op1=Alu.add)
            nc.sync.dma_start(out_d[ms, :], res[:])

    nc.compile()
    return nc


def _host_prep(inputs):
    x = np.ascontiguousarray(inputs["x"], dtype=np.float32)
    W_pts = np.asarray(inputs["W_pts"], dtype=np.float32)
    b_pts = np.asarray(inputs["b_pts"], dtype=np.float32)
    W_comp = np.asarray(inputs["W_comp"], dtype=np.float32)
    b_comp = np.asarray(inputs["b_comp"], dtype=np.float32)

    Wall = np.concatenate([W_pts, W_comp], axis=0)  # (2706, 512)
    WT = np.ascontiguousarray(Wall.T)               # (512, 2706)
    # Row 511 multiplies xc[:,511] == 0, so it is a free slot: plant the
    # bias there (device side plants a ones-row in xh[3] partition 127).
    ball = np.concatenate([b_pts, b_comp]).astype(np.float32)  # (2706,)
    WT[511, :] = ball
    wh = WT.astype(BF)
    wl = (WT - wh.astype(np.float32)).astype(BF)
    # wl row 511 must hold bias-lo vs wh row 511 (xl row is zeroed, and
    # only xh@wh + xh@wl see the ones-row): already exact by construction.
    ident = np.eye(128, dtype=np.float32)
    iota9 = np.tile(np.arange(NSLOTS, dtype=np.float32), (128, 1))
    tri = np.tile(
        (np.arange(8)[None, :] < np.arange(8)[:, None]).astype(np.float32)
        .reshape(1, 64), (128, 1))
    tri = np.ascontiguousarray(tri)

    shared = dict(wh=wh, wl=wl, ident=ident, iota9=iota9, tri=tri,
                  ones_r=None, zeros_r=None)  # filled per-rows in kernel()
    return x, shared


def kernel(**inputs):
    from concourse.bass_utils import run_bass_kernel_spmd

    x, shared = _host_prep(inputs)
    B = x.shape[0]
    rows = B // N_CORES

    key = rows
    if key not in _CACHE:
        _CACHE[key] = _build(rows)
    nc = _CACHE[key]

    shared["ones_r"] = np.ones((1, rows), dtype=BF)
    shared["zeros_r"] = np.zeros((1, rows), dtype=BF)
    in_maps = []
    for c in range(N_CORES):
        m = {"x": x[c * rows:(c + 1) * rows]}
        m.update(shared)
        in_maps.append(m)

    trace = bool(int(os.environ.get("KERNEL_TRACE", "0")))
    r = run_bass_kernel_spmd(nc, in_maps, list(range(N_CORES)), trace=trace)
    kernel.last_exec_time_ns = r.exec_time_ns
    kernel.last_results = r

    out = np.concatenate([r.results[c]["out"] for c in range(N_CORES)], axis=0)
    return out.astype(np.float32)


kernel.last_exec_time_ns = None
kernel.last_results = None


# revision 8
# speedup vs baseline: 1.2075x; 1.2075x over previous
"""Trainium2 Bass kernel for nn_PointEstimator (segment_reduce).

Computation (per batch row b):
    xc = x - x[:, -1:]
    logits = xc @ W_pts.T + b_pts            # (B, 8*336)
    pts    = sort(argmax over each 336-group) # (B, 8)
    params = (xc @ W_comp.T + b_comp)        # (B, 9, 2) slope/intercept
    out[t] = a[s(t)] * t + c[s(t)] + x[:,-1],  s(t) = #(pts <= t)

Device algorithm highlights:
  * batch rows on PSUM partitions; seq (512) is the contraction dim.
  * matmul in a bf16 3-term split (xh@wh + xh@wl + xl@wh) for fp32-level
    logit accuracy (needed: min top-2 logit gap is ~6e-6); bias folded in
    as a K=2 all-ones matmul that opens each PSUM accumulation group.
  * x is transposed on-device via PE transpose; xc=x-lastcol on DVE.
  * argmax without sort: running-max prefix scan (tensor_tensor_scan max)
    then t* = sum_t [rm_t < rm_335] via scalar_tensor_tensor accum (exact
    first-index tie-break).
  * piecewise combine without sort or gather: for each point, its slot
    jump da = a[cnt_le]-a[cnt_lt] (one-hot lookups via broadcast compare),
    deduplicated to the first point of each tied value, scattered with
    gpsimd local_scatter, then out = double prefix-sum:
        u = a0 + cumsum(j1);  res = (c0+lv-a0) + cumsum(u + j2),
        j1[p] = da, j2[p] = da*(p-1)+dc.
"""

import os
import numpy as np
import ml_dtypes

N_CORES = 8
B_FULL = 8192
SEQ = 512
PRED = 336
NPTS = 8
NSLOTS = 9
NOUT = NPTS * PRED + NSLOTS * 2  # 2688 + 18 = 2706
PARAM_OFF = NPTS * PRED          # 2688

BF = ml_dtypes.bfloat16

_CACHE = {}


def _build(rows):
    """Build + compile the per-core Bass program for `rows` batch rows."""
    import concourse.bass as bass
    import concourse.tile as tile
    from concourse import bacc, mybir
    from concourse import library_config
    from contextlib import ExitStack

    f32 = mybir.dt.float32
    bf16 = mybir.dt.bfloat16
    i16 = mybir.dt.int16
    Alu = mybir.AluOpType

    MT = rows // 128
    assert rows % 128 == 0

    nc = bacc.Bacc("TRN2", target_bir_lowering=False, debug=False)

    x_d = nc.dram_tensor("x", [rows, SEQ], f32, kind="ExternalInput").ap()
    wh_d = nc.dram_tensor("wh", [SEQ, NOUT], bf16, kind="ExternalInput").ap()
    wl_d = nc.dram_tensor("wl", [SEQ, NOUT], bf16, kind="ExternalInput").ap()
    ident_d = nc.dram_tensor("ident", [128, 128], f32, kind="ExternalInput").ap()
    iota9_d = nc.dram_tensor("iota9", [128, NSLOTS], f32, kind="ExternalInput").ap()
    tri_d = nc.dram_tensor("tri", [128, 64], f32, kind="ExternalInput").ap()
    ones_r_d = nc.dram_tensor("ones_r", [1, rows], bf16, kind="ExternalInput").ap()
    zeros_r_d = nc.dram_tensor("zeros_r", [1, rows], bf16, kind="ExternalInput").ap()
    out_d = nc.dram_tensor("out", [rows, PRED], f32, kind="ExternalOutput").ap()

    # Logits live in 3 PSUM tiles of 3 groups / 3 groups / 2 groups+params.
    TSPEC = [
        (0, 1008),        # groups 0..2
        (1008, 2016),     # groups 3..5
        (2016, 2706),     # groups 6..7 + 18 params
    ]

    with tile.TileContext(nc) as tc, ExitStack() as ctx:
        const = ctx.enter_context(tc.tile_pool(name="const", bufs=1))
        wpool = ctx.enter_context(tc.tile_pool(name="w", bufs=1))
        xpool = ctx.enter_context(tc.tile_pool(name="xin", bufs=3))
        xs = ctx.enter_context(tc.tile_pool(name="xsplit", bufs=1))
        work = ctx.enter_context(tc.tile_pool(name="work", bufs=2))
        tiny = ctx.enter_context(tc.tile_pool(name="tiny", bufs=2))
        psum = ctx.enter_context(tc.tile_pool(name="psum", bufs=3, space="PSUM"))
        psumT = ctx.enter_context(tc.tile_pool(name="psumT", bufs=2, space="PSUM"))

        nc.gpsimd.load_library(library_config.local_scatter)

        # ---- constants ----
        ident = const.tile([128, 128], f32)
        nc.sync.dma_start(ident[:], ident_d)
        iota9 = const.tile([128, NSLOTS], f32)
        nc.sync.dma_start(iota9[:], iota9_d)
        tri = const.tile([128, 64], f32)
        nc.sync.dma_start(tri[:], tri_d)
        zeros336 = const.tile([128, PRED], f32)
        nc.vector.memset(zeros336[:], 0.0)

        # ---- weights (resident) ----
        whs, wls = [], []
        for kt in range(4):
            t = wpool.tile([128, NOUT], bf16, tag=f"wh{kt}", name=f"wh{kt}")
            nc.sync.dma_start(t[:], wh_d[kt * 128:(kt + 1) * 128, :])
            whs.append(t)
            t = wpool.tile([128, NOUT], bf16, tag=f"wl{kt}", name=f"wl{kt}")
            nc.sync.dma_start(t[:], wl_d[kt * 128:(kt + 1) * 128, :])
            wls.append(t)

        # ---- persistent transposed x splits ----
        xh = [xs.tile([128, rows], bf16, tag=f"xh{kt}", name=f"xh{kt}") for kt in range(4)]
        xl = [xs.tile([128, rows], bf16, tag=f"xl{kt}", name=f"xl{kt}") for kt in range(4)]
        lvs = xs.tile([128, MT], f32, tag="lvs")  # last value per mtile
        # bias rows: xh[3] partition 127 = 1.0 (multiplies W row 511 = bias),
        # xl[3] partition 127 = 0. Written once; per-mtile copies skip row 127.
        nc.sync.dma_start(xh[3][127:128, :], ones_r_d)
        nc.sync.dma_start(xl[3][127:128, :], zeros_r_d)

        # ======== stage A for ALL mtiles: load, xc, transpose, split ====
        for m in range(MT):
            ms = slice(m * 128, (m + 1) * 128)
            xt = xpool.tile([128, SEQ], f32, tag="xt")
            nc.sync.dma_start(xt[:], x_d[ms, :])
            xc = xpool.tile([128, SEQ], f32, tag="xc")
            nc.vector.tensor_scalar(
                out=xc[:], in0=xt[:], scalar1=xt[:, 511:512], scalar2=None,
                op0=Alu.subtract)
            nc.scalar.copy(out=lvs[:, m:m + 1], in_=xt[:, 511:512])

            pst = psumT.tile([128, SEQ], f32, tag="ptrans")
            for kt in range(4):
                ks = slice(kt * 128, (kt + 1) * 128)
                nc.tensor.transpose(pst[:, ks], xc[:, ks], ident[:])
            for kt in range(4):
                ks = slice(kt * 128, (kt + 1) * 128)
                np_ = 127 if kt == 3 else 128
                pp = slice(0, np_)
                nc.scalar.copy(out=xh[kt][pp, ms], in_=pst[pp, ks])
                nc.vector.tensor_tensor(
                    out=xl[kt][pp, ms], in0=pst[pp, ks], in1=xh[kt][pp, ms],
                    op=Alu.subtract)

        for m in range(MT):
            ms = slice(m * 128, (m + 1) * 128)
            # ================= stage B: matmuls =========================
            pts_tiles = []
            for (c0, c1) in TSPEC:
                pt = psum.tile([128, c1 - c0], f32, tag="lg", name=f"lg{c0}", padded_shape=[128, 1024])
                pts_tiles.append(pt)
                w = c1 - c0
                blocks = [(0, min(512, w))]
                if w > 512:
                    blocks.append((512, w))
                terms = [(xh, whs), (xh, wls), (xl, whs)]
                for kt in range(4):
                    for ti, (xop, wop) in enumerate(terms):
                        first = (kt == 0 and ti == 0)
                        last = (kt == 3 and ti == 2)
                        for (b0, b1) in blocks:
                            nc.tensor.matmul(
                                pt[:, b0:b1], xop[kt][:, ms],
                                wop[kt][:, c0 + b0:c0 + b1],
                                start=first, stop=last)

            def lg(col0, col1):
                """AP view of logits columns [col0, col1) across T tiles."""
                for (c0, c1), pt in zip(TSPEC, pts_tiles):
                    if col0 >= c0 and col1 <= c1:
                        return pt[:, col0 - c0:col1 - c0]
                raise AssertionError((col0, col1))

            # params to SBUF (strided copy PSUM->SBUF)
            a_sb = tiny.tile([128, NSLOTS], f32, tag="a_sb")
            c_sb = tiny.tile([128, NSLOTS], f32, tag="c_sb")
            pview = lg(PARAM_OFF, NOUT).rearrange("p (k two) -> p k two", two=2)
            nc.scalar.copy(out=a_sb[:], in_=pview[:, :, 0:1].squeeze(2))
            nc.scalar.copy(out=c_sb[:], in_=pview[:, :, 1:2].squeeze(2))

            # ================= stage C: argmax ==========================
            rm = work.tile([128, NPTS * PRED], f32, tag="rm")
            pts = tiny.tile([128, NPTS], f32, tag="pts")
            for g in range(NPTS):
                gs = slice(g * PRED, (g + 1) * PRED)
                nc.vector.tensor_tensor_scan(
                    out=rm[:, gs], data0=lg(g * PRED, (g + 1) * PRED),
                    data1=zeros336[:], initial=-3.0e38,
                    op0=Alu.max, op1=Alu.bypass)
            cnt_scr = work.tile([128, PRED], bf16, tag="cntscr")
            for g in range(NPTS):
                gs = slice(g * PRED, (g + 1) * PRED)
                # t* = sum_t sign(m - rm_t)  (1 while rm<m, 0 from argmax on)
                nc.scalar.activation(
                    out=cnt_scr[:], in_=rm[:, gs],
                    func=mybir.ActivationFunctionType.Sign,
                    bias=rm[:, g * PRED + 335:g * PRED + 336], scale=-1.0,
                    accum_out=pts[:, g:g + 1])

            # ================= stage D: combine =========================
            p_ap = pts[:]
            p_i = p_ap.unsqueeze(2).broadcast_to([128, NPTS, NPTS])
            p_j = p_ap.unsqueeze(1).broadcast_to([128, NPTS, NPTS])
            LT = tiny.tile([128, 64], f32, tag="LT")
            LE = tiny.tile([128, 64], f32, tag="LE")
            nc.vector.tensor_tensor(
                out=LT[:].rearrange("p (i j) -> p i j", j=8), in0=p_i, in1=p_j,
                op=Alu.is_gt)   # LT[i,j] = p_j < p_i
            nc.vector.tensor_tensor(
                out=LE[:].rearrange("p (i j) -> p i j", j=8), in0=p_i, in1=p_j,
                op=Alu.is_ge)   # LE[i,j] = p_j <= p_i
            cnt_lt = tiny.tile([128, NPTS], f32, tag="cnt_lt")
            cnt_le = tiny.tile([128, NPTS], f32, tag="cnt_le")
            nc.vector.tensor_reduce(
                out=cnt_lt[:], in_=LT[:].rearrange("p (i j) -> p i j", j=8),
                axis=mybir.AxisListType.X, op=Alu.add)
            nc.vector.tensor_reduce(
                out=cnt_le[:], in_=LE[:].rearrange("p (i j) -> p i j", j=8),
                axis=mybir.AxisListType.X, op=Alu.add)
            EQ = tiny.tile([128, 64], f32, tag="EQ")
            nc.vector.tensor_tensor(out=EQ[:], in0=LE[:], in1=LT[:],
                                    op=Alu.subtract)
            EQt = tiny.tile([128, 64], f32, tag="EQt")
            nc.vector.tensor_tensor(out=EQt[:], in0=EQ[:], in1=tri[:],
                                    op=Alu.mult)
            Ecnt = tiny.tile([128, NPTS], f32, tag="Ecnt")
            nc.vector.tensor_reduce(
                out=Ecnt[:], in_=EQt[:].rearrange("p (i j) -> p i j", j=8),
                axis=mybir.AxisListType.X, op=Alu.add)
            isf = tiny.tile([128, NPTS], f32, tag="isf")
            nc.vector.tensor_scalar(out=isf[:], in0=Ecnt[:], scalar1=0.0,
                                    scalar2=None, op0=Alu.is_equal)

            # one-hot lookups a[cnt], c[cnt]
            i9 = iota9[:].unsqueeze(1).broadcast_to([128, NPTS, NSLOTS])
            a_b = a_sb[:].unsqueeze(1).broadcast_to([128, NPTS, NSLOTS])
            c_b = c_sb[:].unsqueeze(1).broadcast_to([128, NPTS, NSLOTS])

            def onehot_lookup(cnt, tag):
                eq = tiny.tile([128, NPTS * NSLOTS], f32, tag=f"eq{tag}", name=f"eq{tag}")
                eqv = eq[:].rearrange("p (i k) -> p i k", k=NSLOTS)
                nc.vector.tensor_tensor(
                    out=eqv, in0=cnt[:].unsqueeze(2).broadcast_to(
                        [128, NPTS, NSLOTS]), in1=i9, op=Alu.is_equal)
                outs = []
                for nm, tbl in (("a", a_b), ("c", c_b)):
                    prod = tiny.tile([128, NPTS * NSLOTS], f32,
                                     tag=f"pr{nm}{tag}", name=f"pr{nm}{tag}")
                    pv = prod[:].rearrange("p (i k) -> p i k", k=NSLOTS)
                    nc.vector.tensor_tensor(out=pv, in0=eqv, in1=tbl,
                                            op=Alu.mult)
                    red = tiny.tile([128, NPTS], f32, tag=f"rd{nm}{tag}", name=f"rd{nm}{tag}")
                    nc.vector.tensor_reduce(out=red[:], in_=pv,
                                            axis=mybir.AxisListType.X,
                                            op=Alu.add)
                    outs.append(red)
                return outs

            a_le, c_le = onehot_lookup(cnt_le, "le")
            a_lt, c_lt = onehot_lookup(cnt_lt, "lt")

            da = tiny.tile([128, NPTS], f32, tag="da")
            dc = tiny.tile([128, NPTS], f32, tag="dc")
            nc.vector.tensor_tensor(out=da[:], in0=a_le[:], in1=a_lt[:],
                                    op=Alu.subtract)
            nc.vector.tensor_tensor(out=dc[:], in0=c_le[:], in1=c_lt[:],
                                    op=Alu.subtract)
            # j2 = da*(p-1) + dc
            j2a = tiny.tile([128, NPTS], f32, tag="j2a")
            nc.vector.scalar_tensor_tensor(
                out=j2a[:], in0=pts[:], scalar=1.0, in1=da[:],
                op0=Alu.subtract, op1=Alu.mult)
            j2v = tiny.tile([128, NPTS], f32, tag="j2v")
            nc.vector.tensor_tensor(out=j2v[:], in0=j2a[:], in1=dc[:],
                                    op=Alu.add)
            # scatter index: is_first ? p : -1
            p1f = tiny.tile([128, NPTS], f32, tag="p1f")
            nc.vector.scalar_tensor_tensor(
                out=p1f[:], in0=pts[:], scalar=1.0, in1=isf[:],
                op0=Alu.add, op1=Alu.mult)
            sidxf = tiny.tile([128, NPTS], f32, tag="sidxf")
            nc.vector.tensor_scalar(out=sidxf[:], in0=p1f[:], scalar1=1.0,
                                    scalar2=None, op0=Alu.subtract)
            sidx = tiny.tile([128, NPTS], i16, tag="sidx")
            nc.vector.tensor_copy(out=sidx[:], in_=sidxf[:])

            # bf16 hi/lo split of scatter payloads
            def bsplit(src, tag):
                hi = tiny.tile([128, NPTS], bf16, tag=f"{tag}h", name=f"{tag}h")
                nc.vector.tensor_copy(out=hi[:], in_=src[:])
                lo = tiny.tile([128, NPTS], bf16, tag=f"{tag}l", name=f"{tag}l")
                nc.vector.tensor_tensor(out=lo[:], in0=src[:], in1=hi[:],
                                        op=Alu.subtract)
                return hi, lo

            j1h, j1l = bsplit(da, "j1")
            j2h, j2l = bsplit(j2v, "j2")

            scat = []
            for nm, payload in (("s1h", j1h), ("s1l", j1l),
                                ("s2h", j2h), ("s2l", j2l)):
                s = work.tile([128, PRED], bf16, tag=nm, name=nm)
                nc.gpsimd.local_scatter(
                    out_ap=s[:], data_ap=payload[:], idxs_ap=sidx[:],
                    channels=128, num_elems=PRED, num_idxs=NPTS)
                scat.append(s)

            u = work.tile([128, PRED], f32, tag="u")
            nc.vector.tensor_tensor_scan(
                out=u[:], data0=scat[0][:], data1=scat[1][:],
                initial=a_sb[:, 0:1], op0=Alu.add, op1=Alu.add)
            j2s = work.tile([128, PRED], f32, tag="j2s")
            nc.vector.tensor_tensor(out=j2s[:], in0=scat[2][:],
                                    in1=scat[3][:], op=Alu.add)
            init2 = tiny.tile([128, 1], f32, tag="init2")
            nc.vector.scalar_tensor_tensor(
                out=init2[:], in0=c_sb[:, 0:1], scalar=a_sb[:, 0:1],
                in1=lvs[:, m:m + 1], op0=Alu.subtract, op1=Alu.add)
            res = work.tile([128, PRED], f32, tag="res")
            nc.vector.tensor_tensor_scan(
                out=res[:], data0=u[:], data1=j2s[:],
                initial=init2[:], op0=Alu.add, op1=Alu.add)
            nc.sync.dma_start(out_d[ms, :], res[:])

    nc.compile()
    return nc


def _host_prep(inputs):
    x = np.ascontiguousarray(inputs["x"], dtype=np.float32)
    W_pts = np.asarray(inputs["W_pts"], dtype=np.float32)
    b_pts = np.asarray(inputs["b_pts"], dtype=np.float32)
    W_comp = np.asarray(inputs["W_comp"], dtype=np.float32)
    b_comp = np.asarray(inputs["b_comp"], dtype=np.float32)

    Wall = np.concatenate([W_pts, W_comp], axis=0)  # (2706, 512)
    WT = np.ascontiguousarray(Wall.T)               # (512, 2706)
    # Row 511 multiplies xc[:,511] == 0, so it is a free slot: plant the
    # bias there (device side plants a ones-row in xh[3] partition 127).
    ball = np.concatenate([b_pts, b_comp]).astype(np.float32)  # (2706,)
    WT[511, :] = ball
    wh = WT.astype(BF)
    wl = (WT - wh.astype(np.float32)).astype(BF)
    # wl row 511 must hold bias-lo vs wh row 511 (xl row is zeroed, and
    # only xh@wh + xh@wl see the ones-row): already exact by construction.
    ident = np.eye(128, dtype=np.float32)
    iota9 = np.tile(np.arange(NSLOTS, dtype=np.float32), (128, 1))
    tri = np.tile(
        (np.arange(8)[None, :] < np.arange(8)[:, None]).astype(np.float32)
        .reshape(1, 64), (128, 1))
    tri = np.ascontiguousarray(tri)

    shared = dict(wh=wh, wl=wl, ident=ident, iota9=iota9, tri=tri,
                  ones_r=None, zeros_r=None)  # filled per-rows in kernel()
    return x, shared


def kernel(**inputs):
    from concourse.bass_utils import run_bass_kernel_spmd

    x, shared = _host_prep(inputs)
    B = x.shape[0]
    rows = B // N_CORES

    key = rows
    if key not in _CACHE:
        _CACHE[key] = _build(rows)
    nc = _CACHE[key]

    shared["ones_r"] = np.ones((1, rows), dtype=BF)
    shared["zeros_r"] = np.zeros((1, rows), dtype=BF)
    in_maps = []
    for c in range(N_CORES):
        m = {"x": x[c * rows:(c + 1) * rows]}
        m.update(shared)
        in_maps.append(m)

    trace = bool(int(os.environ.get("KERNEL_TRACE", "0")))
    r = run_bass_kernel_spmd(nc, in_maps, list(range(N_CORES)), trace=trace)
    kernel.last_exec_time_ns = r.exec_time_ns
    kernel.last_results = r

    out = np.concatenate([r.results[c]["out"] for c in range(N_CORES)], axis=0)
    return out.astype(np.float32)


kernel.last_exec_time_ns = None
kernel.last_results = None


# revision 9
# speedup vs baseline: 1.2377x; 1.0250x over previous
"""Trainium2 Bass kernel for nn_PointEstimator (segment_reduce).

Computation (per batch row b):
    xc = x - x[:, -1:]
    logits = xc @ W_pts.T + b_pts            # (B, 8*336)
    pts    = sort(argmax over each 336-group) # (B, 8)
    params = (xc @ W_comp.T + b_comp)        # (B, 9, 2) slope/intercept
    out[t] = a[s(t)] * t + c[s(t)] + x[:,-1],  s(t) = #(pts <= t)

Device algorithm highlights:
  * batch rows on PSUM partitions; seq (512) is the contraction dim.
  * matmul in a bf16 3-term split (xh@wh + xh@wl + xl@wh) for fp32-level
    logit accuracy (needed: min top-2 logit gap is ~6e-6); bias folded in
    as a K=2 all-ones matmul that opens each PSUM accumulation group.
  * x is transposed on-device via PE transpose; xc=x-lastcol on DVE.
  * argmax without sort: running-max prefix scan (tensor_tensor_scan max)
    then t* = sum_t [rm_t < rm_335] via scalar_tensor_tensor accum (exact
    first-index tie-break).
  * piecewise combine without sort or gather: for each point, its slot
    jump da = a[cnt_le]-a[cnt_lt] (one-hot lookups via broadcast compare),
    deduplicated to the first point of each tied value, scattered with
    gpsimd local_scatter, then out = double prefix-sum:
        u = a0 + cumsum(j1);  res = (c0+lv-a0) + cumsum(u + j2),
        j1[p] = da, j2[p] = da*(p-1)+dc.
"""

import os
import numpy as np
import ml_dtypes

N_CORES = 8
B_FULL = 8192
SEQ = 512
PRED = 336
NPTS = 8
NSLOTS = 9
NOUT = NPTS * PRED + NSLOTS * 2  # 2688 + 18 = 2706
PARAM_OFF = NPTS * PRED          # 2688

BF = ml_dtypes.bfloat16

_CACHE = {}


def _build(rows):
    """Build + compile the per-core Bass program for `rows` batch rows."""
    import concourse.bass as bass
    import concourse.tile as tile
    from concourse import bacc, mybir
    from concourse import library_config
    from contextlib import ExitStack

    f32 = mybir.dt.float32
    bf16 = mybir.dt.bfloat16
    i16 = mybir.dt.int16
    Alu = mybir.AluOpType

    MT = rows // 128
    assert rows % 128 == 0

    nc = bacc.Bacc("TRN2", target_bir_lowering=False, debug=False)

    x_d = nc.dram_tensor("x", [rows, SEQ], f32, kind="ExternalInput").ap()
    wh_d = nc.dram_tensor("wh", [SEQ, NOUT], bf16, kind="ExternalInput").ap()
    wl_d = nc.dram_tensor("wl", [SEQ, NOUT], bf16, kind="ExternalInput").ap()
    ident_d = nc.dram_tensor("ident", [128, 128], f32, kind="ExternalInput").ap()
    iota9_d = nc.dram_tensor("iota9", [128, NSLOTS], f32, kind="ExternalInput").ap()
    tri_d = nc.dram_tensor("tri", [128, 64], f32, kind="ExternalInput").ap()
    ones_r_d = nc.dram_tensor("ones_r", [1, rows], bf16, kind="ExternalInput").ap()
    zeros_r_d = nc.dram_tensor("zeros_r", [1, rows], bf16, kind="ExternalInput").ap()
    out_d = nc.dram_tensor("out", [rows, PRED], f32, kind="ExternalOutput").ap()

    # Logits live in 3 PSUM tiles of 3 groups / 3 groups / 2 groups+params.
    TSPEC = [
        (0, 1008),        # groups 0..2
        (1008, 2016),     # groups 3..5
        (2016, 2706),     # groups 6..7 + 18 params
    ]

    with tile.TileContext(nc) as tc, ExitStack() as ctx:
        const = ctx.enter_context(tc.tile_pool(name="const", bufs=1))
        wpool = ctx.enter_context(tc.tile_pool(name="w", bufs=1))
        xpool = ctx.enter_context(tc.tile_pool(name="xin", bufs=3))
        xs = ctx.enter_context(tc.tile_pool(name="xsplit", bufs=1))
        work = ctx.enter_context(tc.tile_pool(name="work", bufs=2))
        tiny = ctx.enter_context(tc.tile_pool(name="tiny", bufs=2))
        psum = ctx.enter_context(tc.tile_pool(name="psum", bufs=3, space="PSUM"))
        psumT = ctx.enter_context(tc.tile_pool(name="psumT", bufs=2, space="PSUM"))

        nc.gpsimd.load_library(library_config.local_scatter)

        # ---- constants needed by stage A first (x DMAs must win BW) ----
        ident = const.tile([128, 128], f32)
        nc.sync.dma_start(ident[:], ident_d)
        zeros336 = const.tile([128, PRED], f32)
        nc.vector.memset(zeros336[:], 0.0)

        # ---- persistent transposed x splits ----
        xh = [xs.tile([128, rows], bf16, tag=f"xh{kt}", name=f"xh{kt}") for kt in range(4)]
        xl = [xs.tile([128, rows], bf16, tag=f"xl{kt}", name=f"xl{kt}") for kt in range(4)]
        lvs = xs.tile([128, MT], f32, tag="lvs")  # last value per mtile
        # bias rows: xh[3] partition 127 = 1.0 (multiplies W row 511 = bias),
        # xl[3] partition 127 = 0. Written once; per-mtile copies skip row 127.
        nc.sync.dma_start(xh[3][127:128, :], ones_r_d)
        nc.sync.dma_start(xl[3][127:128, :], zeros_r_d)

        # ======== stage A for ALL mtiles: load, xc, transpose, split ====
        for m in range(MT):
            ms = slice(m * 128, (m + 1) * 128)
            xt = xpool.tile([128, SEQ], f32, tag="xt")
            nc.sync.dma_start(xt[:], x_d[ms, :])
            xc = xpool.tile([128, SEQ], f32, tag="xc")
            nc.vector.tensor_scalar(
                out=xc[:], in0=xt[:], scalar1=xt[:, 511:512], scalar2=None,
                op0=Alu.subtract)
            nc.scalar.copy(out=lvs[:, m:m + 1], in_=xt[:, 511:512])

            pst = psumT.tile([128, SEQ], f32, tag="ptrans")
            for kt in range(4):
                ks = slice(kt * 128, (kt + 1) * 128)
                nc.tensor.transpose(pst[:, ks], xc[:, ks], ident[:])
            for kt in range(4):
                ks = slice(kt * 128, (kt + 1) * 128)
                np_ = 127 if kt == 3 else 128
                pp = slice(0, np_)
                nc.scalar.copy(out=xh[kt][pp, ms], in_=pst[pp, ks])
                nc.vector.tensor_tensor(
                    out=xl[kt][pp, ms], in0=pst[pp, ks], in1=xh[kt][pp, ms],
                    op=Alu.subtract)

        # ---- weights (resident; emitted after x so wh0/wl0 land first) ----
        whs, wls = [], []
        for kt in range(4):
            t = wpool.tile([128, NOUT], bf16, tag=f"wh{kt}", name=f"wh{kt}")
            nc.sync.dma_start(t[:], wh_d[kt * 128:(kt + 1) * 128, :])
            whs.append(t)
            t = wpool.tile([128, NOUT], bf16, tag=f"wl{kt}", name=f"wl{kt}")
            nc.sync.dma_start(t[:], wl_d[kt * 128:(kt + 1) * 128, :])
            wls.append(t)
        iota9 = const.tile([128, NSLOTS], f32)
        nc.sync.dma_start(iota9[:], iota9_d)
        tri = const.tile([128, 64], f32)
        nc.sync.dma_start(tri[:], tri_d)

        for m in range(MT):
            ms = slice(m * 128, (m + 1) * 128)
            # ================= stage B: matmuls =========================
            pts_tiles = []
            tblocks = []
            for (c0, c1) in TSPEC:
                pt = psum.tile([128, c1 - c0], f32, tag="lg", name=f"lg{c0}", padded_shape=[128, 1024])
                pts_tiles.append(pt)
                w = c1 - c0
                blocks = [(0, min(512, w))]
                if w > 512:
                    blocks.append((512, w))
                tblocks.append(blocks)
            terms = [(xh, whs), (xh, wls), (xl, whs)]
            if m == 0:
                # kt-outer: first matmuls need only the kt=0 weight tiles,
                # which stream in while these run (shorter cold preamble).
                for kt in range(4):
                    for ti, (xop, wop) in enumerate(terms):
                        first = (kt == 0 and ti == 0)
                        last = (kt == 3 and ti == 2)
                        for (c0, c1), pt, blocks in zip(TSPEC, pts_tiles,
                                                        tblocks):
                            for (b0, b1) in blocks:
                                nc.tensor.matmul(
                                    pt[:, b0:b1], xop[kt][:, ms],
                                    wop[kt][:, c0 + b0:c0 + b1],
                                    start=first, stop=last)
            else:
                # T-outer: finish each PSUM tile early so its scans release it
                # while the next tile still matmuls.
                for (c0, c1), pt, blocks in zip(TSPEC, pts_tiles, tblocks):
                    for kt in range(4):
                        for ti, (xop, wop) in enumerate(terms):
                            first = (kt == 0 and ti == 0)
                            last = (kt == 3 and ti == 2)
                            for (b0, b1) in blocks:
                                nc.tensor.matmul(
                                    pt[:, b0:b1], xop[kt][:, ms],
                                    wop[kt][:, c0 + b0:c0 + b1],
                                    start=first, stop=last)

            def lg(col0, col1):
                """AP view of logits columns [col0, col1) across T tiles."""
                for (c0, c1), pt in zip(TSPEC, pts_tiles):
                    if col0 >= c0 and col1 <= c1:
                        return pt[:, col0 - c0:col1 - c0]
                raise AssertionError((col0, col1))

            # params to SBUF (strided copy PSUM->SBUF)
            a_sb = tiny.tile([128, NSLOTS], f32, tag="a_sb")
            c_sb = tiny.tile([128, NSLOTS], f32, tag="c_sb")
            pview = lg(PARAM_OFF, NOUT).rearrange("p (k two) -> p k two", two=2)
            nc.scalar.copy(out=a_sb[:], in_=pview[:, :, 0:1].squeeze(2))
            nc.scalar.copy(out=c_sb[:], in_=pview[:, :, 1:2].squeeze(2))

            # ================= stage C: argmax ==========================
            rm = work.tile([128, NPTS * PRED], f32, tag="rm")
            pts = tiny.tile([128, NPTS], f32, tag="pts")
            for g in range(NPTS):
                gs = slice(g * PRED, (g + 1) * PRED)
                nc.vector.tensor_tensor_scan(
                    out=rm[:, gs], data0=lg(g * PRED, (g + 1) * PRED),
                    data1=zeros336[:], initial=-3.0e38,
                    op0=Alu.max, op1=Alu.bypass)
            cnt_scr = work.tile([128, PRED], bf16, tag="cntscr")
            for g in range(NPTS):
                gs = slice(g * PRED, (g + 1) * PRED)
                # t* = sum_t sign(m - rm_t)  (1 while rm<m, 0 from argmax on)
                nc.scalar.activation(
                    out=cnt_scr[:], in_=rm[:, gs],
                    func=mybir.ActivationFunctionType.Sign,
                    bias=rm[:, g * PRED + 335:g * PRED + 336], scale=-1.0,
                    accum_out=pts[:, g:g + 1])

            # ================= stage D: combine =========================
            p_ap = pts[:]
            p_i = p_ap.unsqueeze(2).broadcast_to([128, NPTS, NPTS])
            p_j = p_ap.unsqueeze(1).broadcast_to([128, NPTS, NPTS])
            LT = tiny.tile([128, 64], f32, tag="LT")
            LE = tiny.tile([128, 64], f32, tag="LE")
            nc.vector.tensor_tensor(
                out=LT[:].rearrange("p (i j) -> p i j", j=8), in0=p_i, in1=p_j,
                op=Alu.is_gt)   # LT[i,j] = p_j < p_i
            nc.vector.tensor_tensor(
                out=LE[:].rearrange("p (i j) -> p i j", j=8), in0=p_i, in1=p_j,
                op=Alu.is_ge)   # LE[i,j] = p_j <= p_i
            cnt_lt = tiny.tile([128, NPTS], f32, tag="cnt_lt")
            cnt_le = tiny.tile([128, NPTS], f32, tag="cnt_le")
            nc.vector.tensor_reduce(
                out=cnt_lt[:], in_=LT[:].rearrange("p (i j) -> p i j", j=8),
                axis=mybir.AxisListType.X, op=Alu.add)
            nc.vector.tensor_reduce(
                out=cnt_le[:], in_=LE[:].rearrange("p (i j) -> p i j", j=8),
                axis=mybir.AxisListType.X, op=Alu.add)
            EQ = tiny.tile([128, 64], f32, tag="EQ")
            nc.vector.tensor_tensor(out=EQ[:], in0=LE[:], in1=LT[:],
                                    op=Alu.subtract)
            EQt = tiny.tile([128, 64], f32, tag="EQt")
            nc.vector.tensor_tensor(out=EQt[:], in0=EQ[:], in1=tri[:],
                                    op=Alu.mult)
            Ecnt = tiny.tile([128, NPTS], f32, tag="Ecnt")
            nc.vector.tensor_reduce(
                out=Ecnt[:], in_=EQt[:].rearrange("p (i j) -> p i j", j=8),
                axis=mybir.AxisListType.X, op=Alu.add)
            isf = tiny.tile([128, NPTS], f32, tag="isf")
            nc.vector.tensor_scalar(out=isf[:], in0=Ecnt[:], scalar1=0.0,
                                    scalar2=None, op0=Alu.is_equal)

            # one-hot lookups a[cnt], c[cnt]
            i9 = iota9[:].unsqueeze(1).broadcast_to([128, NPTS, NSLOTS])
            a_b = a_sb[:].unsqueeze(1).broadcast_to([128, NPTS, NSLOTS])
            c_b = c_sb[:].unsqueeze(1).broadcast_to([128, NPTS, NSLOTS])

            def onehot_lookup(cnt, tag):
                eq = tiny.tile([128, NPTS * NSLOTS], f32, tag=f"eq{tag}", name=f"eq{tag}")
                eqv = eq[:].rearrange("p (i k) -> p i k", k=NSLOTS)
                nc.vector.tensor_tensor(
                    out=eqv, in0=cnt[:].unsqueeze(2).broadcast_to(
                        [128, NPTS, NSLOTS]), in1=i9, op=Alu.is_equal)
                outs = []
                for nm, tbl in (("a", a_b), ("c", c_b)):
                    prod = tiny.tile([128, NPTS * NSLOTS], f32,
                                     tag=f"pr{nm}{tag}", name=f"pr{nm}{tag}")
                    pv = prod[:].rearrange("p (i k) -> p i k", k=NSLOTS)
                    nc.vector.tensor_tensor(out=pv, in0=eqv, in1=tbl,
                                            op=Alu.mult)
                    red = tiny.tile([128, NPTS], f32, tag=f"rd{nm}{tag}", name=f"rd{nm}{tag}")
                    nc.vector.tensor_reduce(out=red[:], in_=pv,
                                            axis=mybir.AxisListType.X,
                                            op=Alu.add)
                    outs.append(red)
                return outs

            a_le, c_le = onehot_lookup(cnt_le, "le")
            a_lt, c_lt = onehot_lookup(cnt_lt, "lt")

            da = tiny.tile([128, NPTS], f32, tag="da")
            dc = tiny.tile([128, NPTS], f32, tag="dc")
            nc.vector.tensor_tensor(out=da[:], in0=a_le[:], in1=a_lt[:],
                                    op=Alu.subtract)
            nc.vector.tensor_tensor(out=dc[:], in0=c_le[:], in1=c_lt[:],
                                    op=Alu.subtract)
            # j2 = da*(p-1) + dc
            j2a = tiny.tile([128, NPTS], f32, tag="j2a")
            nc.vector.scalar_tensor_tensor(
                out=j2a[:], in0=pts[:], scalar=1.0, in1=da[:],
                op0=Alu.subtract, op1=Alu.mult)
            j2v = tiny.tile([128, NPTS], f32, tag="j2v")
            nc.vector.tensor_tensor(out=j2v[:], in0=j2a[:], in1=dc[:],
                                    op=Alu.add)
            # scatter index: is_first ? p : -1
            p1f = tiny.tile([128, NPTS], f32, tag="p1f")
            nc.vector.scalar_tensor_tensor(
                out=p1f[:], in0=pts[:], scalar=1.0, in1=isf[:],
                op0=Alu.add, op1=Alu.mult)
            sidxf = tiny.tile([128, NPTS], f32, tag="sidxf")
            nc.vector.tensor_scalar(out=sidxf[:], in0=p1f[:], scalar1=1.0,
                                    scalar2=None, op0=Alu.subtract)
            sidx = tiny.tile([128, NPTS], i16, tag="sidx")
            nc.vector.tensor_copy(out=sidx[:], in_=sidxf[:])

            # bf16 hi/lo split of scatter payloads
            def bsplit(src, tag):
                hi = tiny.tile([128, NPTS], bf16, tag=f"{tag}h", name=f"{tag}h")
                nc.vector.tensor_copy(out=hi[:], in_=src[:])
                lo = tiny.tile([128, NPTS], bf16, tag=f"{tag}l", name=f"{tag}l")
                nc.vector.tensor_tensor(out=lo[:], in0=src[:], in1=hi[:],
                                        op=Alu.subtract)
                return hi, lo

            j1h, j1l = bsplit(da, "j1")
            j2h, j2l = bsplit(j2v, "j2")

            scat = []
            for nm, payload in (("s1h", j1h), ("s1l", j1l),
                                ("s2h", j2h), ("s2l", j2l)):
                s = work.tile([128, PRED], bf16, tag=nm, name=nm)
                nc.gpsimd.local_scatter(
                    out_ap=s[:], data_ap=payload[:], idxs_ap=sidx[:],
                    channels=128, num_elems=PRED, num_idxs=NPTS)
                scat.append(s)

            u = work.tile([128, PRED], f32, tag="u")
            nc.vector.tensor_tensor_scan(
                out=u[:], data0=scat[0][:], data1=scat[1][:],
                initial=a_sb[:, 0:1], op0=Alu.add, op1=Alu.add)
            j2s = work.tile([128, PRED], f32, tag="j2s")
            nc.vector.tensor_tensor(out=j2s[:], in0=scat[2][:],
                                    in1=scat[3][:], op=Alu.add)
            init2 = tiny.tile([128, 1], f32, tag="init2")
            nc.vector.scalar_tensor_tensor(
                out=init2[:], in0=c_sb[:, 0:1], scalar=a_sb[:, 0:1],
                in1=lvs[:, m:m + 1], op0=Alu.subtract, op1=Alu.add)
            res = work.tile([128, PRED], f32, tag="res")
            nc.vector.tensor_tensor_scan(
                out=res[:], data0=u[:], data1=j2s[:],
                initial=init2[:], op0=Alu.add, op1=Alu.add)
            nc.sync.dma_start(out_d[ms, :], res[:])

    nc.compile()
    return nc


def _host_prep(inputs):
    x = np.ascontiguousarray(inputs["x"], dtype=np.float32)
    W_pts = np.asarray(inputs["W_pts"], dtype=np.float32)
    b_pts = np.asarray(inputs["b_pts"], dtype=np.float32)
    W_comp = np.asarray(inputs["W_comp"], dtype=np.float32)
    b_comp = np.asarray(inputs["b_comp"], dtype=np.float32)

    Wall = np.concatenate([W_pts, W_comp], axis=0)  # (2706, 512)
    WT = np.ascontiguousarray(Wall.T)               # (512, 2706)
    # Row 511 multiplies xc[:,511] == 0, so it is a free slot: plant the
    # bias there (device side plants a ones-row in xh[3] partition 127).
    ball = np.concatenate([b_pts, b_comp]).astype(np.float32)  # (2706,)
    WT[511, :] = ball
    wh = WT.astype(BF)
    wl = (WT - wh.astype(np.float32)).astype(BF)
    # wl row 511 must hold bias-lo vs wh row 511 (xl row is zeroed, and
    # only xh@wh + xh@wl see the ones-row): already exact by construction.
    ident = np.eye(128, dtype=np.float32)
    iota9 = np.tile(np.arange(NSLOTS, dtype=np.float32), (128, 1))
    tri = np.tile(
        (np.arange(8)[None, :] < np.arange(8)[:, None]).astype(np.float32)
        .reshape(1, 64), (128, 1))
    tri = np.ascontiguousarray(tri)

    shared = dict(wh=wh, wl=wl, ident=ident, iota9=iota9, tri=tri,
                  ones_r=None, zeros_r=None)  # filled per-rows in kernel()
    return x, shared


def kernel(**inputs):
    from concourse.bass_utils import run_bass_kernel_spmd

    x, shared = _host_prep(inputs)
    B = x.shape[0]
    rows = B // N_CORES

    key = rows
    if key not in _CACHE:
        _CACHE[key] = _build(rows)
    nc = _CACHE[key]

    shared["ones_r"] = np.ones((1, rows), dtype=BF)
    shared["zeros_r"] = np.zeros((1, rows), dtype=BF)
    in_maps = []
    for c in range(N_CORES):
        m = {"x": x[c * rows:(c + 1) * rows]}
        m.update(shared)
        in_maps.append(m)

    trace = bool(int(os.environ.get("KERNEL_TRACE", "0")))
    r = run_bass_kernel_spmd(nc, in_maps, list(range(N_CORES)), trace=trace)
    kernel.last_exec_time_ns = r.exec_time_ns
    kernel.last_results = r

    out = np.concatenate([r.results[c]["out"] for c in range(N_CORES)], axis=0)
    return out.astype(np.float32)


kernel.last_exec_time_ns = None
kernel.last_results = None


# revision 11
# speedup vs baseline: 1.2541x; 1.0133x over previous
"""Trainium2 Bass kernel for nn_PointEstimator (segment_reduce).

Computation (per batch row b):
    xc = x - x[:, -1:]
    logits = xc @ W_pts.T + b_pts            # (B, 8*336)
    pts    = sort(argmax over each 336-group) # (B, 8)
    params = (xc @ W_comp.T + b_comp)        # (B, 9, 2) slope/intercept
    out[t] = a[s(t)] * t + c[s(t)] + x[:,-1],  s(t) = #(pts <= t)

Device algorithm highlights:
  * batch rows on PSUM partitions; seq (512) is the contraction dim.
  * matmul in a bf16 3-term split (xh@wh + xh@wl + xl@wh) for fp32-level
    logit accuracy (needed: min top-2 logit gap is ~6e-6); bias folded in
    as a K=2 all-ones matmul that opens each PSUM accumulation group.
  * x is transposed on-device via PE transpose; xc=x-lastcol on DVE.
  * argmax without sort: running-max prefix scan (tensor_tensor_scan max)
    then t* = sum_t [rm_t < rm_335] via scalar_tensor_tensor accum (exact
    first-index tie-break).
  * piecewise combine without sort or gather: for each point, its slot
    jump da = a[cnt_le]-a[cnt_lt] (one-hot lookups via broadcast compare),
    deduplicated to the first point of each tied value, scattered with
    gpsimd local_scatter, then out = double prefix-sum:
        u = a0 + cumsum(j1);  res = (c0+lv-a0) + cumsum(u + j2),
        j1[p] = da, j2[p] = da*(p-1)+dc.
"""

import os
import numpy as np
import ml_dtypes

N_CORES = 8
B_FULL = 8192
SEQ = 512
PRED = 336
NPTS = 8
NSLOTS = 9
NOUT = NPTS * PRED + NSLOTS * 2  # 2688 + 18 = 2706
PARAM_OFF = NPTS * PRED          # 2688

BF = ml_dtypes.bfloat16

_CACHE = {}


def _build(rows):
    """Build + compile the per-core Bass program for `rows` batch rows."""
    import concourse.bass as bass
    import concourse.tile as tile
    from concourse import bacc, mybir
    from concourse import library_config
    from contextlib import ExitStack

    f32 = mybir.dt.float32
    bf16 = mybir.dt.bfloat16
    i16 = mybir.dt.int16
    Alu = mybir.AluOpType

    MT = rows // 128
    assert rows % 128 == 0

    nc = bacc.Bacc("TRN2", target_bir_lowering=False, debug=False)

    x_d = nc.dram_tensor("x", [rows, SEQ], f32, kind="ExternalInput").ap()
    wh_d = nc.dram_tensor("wh", [SEQ, NOUT], bf16, kind="ExternalInput").ap()
    wl_d = nc.dram_tensor("wl", [SEQ, NOUT], bf16, kind="ExternalInput").ap()
    ident_d = nc.dram_tensor("ident", [128, 128], f32, kind="ExternalInput").ap()
    iota9_d = nc.dram_tensor("iota9", [128, NSLOTS], f32, kind="ExternalInput").ap()
    tri_d = nc.dram_tensor("tri", [128, 64], f32, kind="ExternalInput").ap()
    ones_r_d = nc.dram_tensor("ones_r", [1, rows], bf16, kind="ExternalInput").ap()
    zeros_r_d = nc.dram_tensor("zeros_r", [1, rows], bf16, kind="ExternalInput").ap()
    out_d = nc.dram_tensor("out", [rows, PRED], f32, kind="ExternalOutput").ap()

    # Logits live in 3 PSUM tiles of 3 groups / 3 groups / 2 groups+params.
    TSPEC = [
        (0, 1008),        # groups 0..2
        (1008, 2016),     # groups 3..5
        (2016, 2706),     # groups 6..7 + 18 params
    ]

    with tile.TileContext(nc) as tc, ExitStack() as ctx:
        const = ctx.enter_context(tc.tile_pool(name="const", bufs=1))
        wpool = ctx.enter_context(tc.tile_pool(name="w", bufs=1))
        xpool = ctx.enter_context(tc.tile_pool(name="xin", bufs=3))
        xs = ctx.enter_context(tc.tile_pool(name="xsplit", bufs=1))
        work = ctx.enter_context(tc.tile_pool(name="work", bufs=2))
        tiny = ctx.enter_context(tc.tile_pool(name="tiny", bufs=2))
        psum = ctx.enter_context(tc.tile_pool(name="psum", bufs=3, space="PSUM"))
        psumT = ctx.enter_context(tc.tile_pool(name="psumT", bufs=2, space="PSUM"))

        nc.gpsimd.load_library(library_config.local_scatter)

        # ---- constants needed by stage A first (x DMAs must win BW) ----
        ident = const.tile([128, 128], f32)
        nc.sync.dma_start(ident[:], ident_d)
        zeros336 = const.tile([128, PRED], f32)
        nc.vector.memset(zeros336[:], 0.0)

        # ---- persistent transposed x splits ----
        xh = [xs.tile([128, rows], bf16, tag=f"xh{kt}", name=f"xh{kt}") for kt in range(4)]
        xl = [xs.tile([128, rows], bf16, tag=f"xl{kt}", name=f"xl{kt}") for kt in range(4)]
        lvs = xs.tile([128, MT], f32, tag="lvs")  # last value per mtile
        # bias rows: xh[3] partition 127 = 1.0 (multiplies W row 511 = bias),
        # xl[3] partition 127 = 0. Written once; per-mtile copies skip row 127.
        nc.sync.dma_start(xh[3][127:128, :], ones_r_d)
        nc.sync.dma_start(xl[3][127:128, :], zeros_r_d)

        # ======== stage A for ALL mtiles: load, xc, transpose, split ====
        whs, wls = [None] * 4, [None] * 4

        def dma_w(kt):
            t = wpool.tile([128, NOUT], bf16, tag=f"wh{kt}", name=f"wh{kt}")
            nc.sync.dma_start(t[:], wh_d[kt * 128:(kt + 1) * 128, :])
            whs[kt] = t
            t = wpool.tile([128, NOUT], bf16, tag=f"wl{kt}", name=f"wl{kt}")
            nc.sync.dma_start(t[:], wl_d[kt * 128:(kt + 1) * 128, :])
            wls[kt] = t

        for m in range(MT):
            ms = slice(m * 128, (m + 1) * 128)
            xt = xpool.tile([128, SEQ], f32, tag="xt")
            nc.sync.dma_start(xt[:], x_d[ms, :])
            xc = xpool.tile([128, SEQ], f32, tag="xc")
            nc.vector.tensor_scalar(
                out=xc[:], in0=xt[:], scalar1=xt[:, 511:512], scalar2=None,
                op0=Alu.subtract)
            nc.scalar.copy(out=lvs[:, m:m + 1], in_=xt[:, 511:512])

            pst = psumT.tile([128, SEQ], f32, tag="ptrans")
            for kt in range(4):
                ks = slice(kt * 128, (kt + 1) * 128)
                nc.tensor.transpose(pst[:, ks], xc[:, ks], ident[:])
            for kt in range(4):
                ks = slice(kt * 128, (kt + 1) * 128)
                np_ = 127 if kt == 3 else 128
                pp = slice(0, np_)
                nc.scalar.copy(out=xh[kt][pp, ms], in_=pst[pp, ks])
                nc.vector.tensor_tensor(
                    out=xl[kt][pp, ms], in0=pst[pp, ks], in1=xh[kt][pp, ms],
                    op=Alu.subtract)
            if m == 0:
                dma_w(0)   # kt=0 weights first: m=0 matmuls are kt-outer

        for kt in range(1, 4):
            dma_w(kt)
        iota9 = const.tile([128, NSLOTS], f32)
        nc.sync.dma_start(iota9[:], iota9_d)
        tri = const.tile([128, 64], f32)
        nc.sync.dma_start(tri[:], tri_d)

        for m in range(MT):
            ms = slice(m * 128, (m + 1) * 128)
            # ================= stage B: matmuls =========================
            tspec_m = TSPEC
            pts_tiles = []
            tblocks = []
            for (c0, c1) in tspec_m:
                pt = psum.tile([128, c1 - c0], f32, tag="lg", name=f"lg{c0}", padded_shape=[128, 1024])
                pts_tiles.append(pt)
                w = c1 - c0
                blocks = [(0, min(512, w))]
                if w > 512:
                    blocks.append((512, w))
                tblocks.append(blocks)
            terms = ([(xh, whs), (xl, whs), (xh, wls)] if m == 0
                     else [(xh, whs), (xh, wls), (xl, whs)])
            if m == 0:
                # kt-outer: first matmuls need only the kt=0 weight tiles,
                # which stream in while these run (shorter cold preamble).
                for kt in range(4):
                    for ti, (xop, wop) in enumerate(terms):
                        first = (kt == 0 and ti == 0)
                        last = (kt == 3 and ti == 2)
                        for (c0, c1), pt, blocks in zip(tspec_m, pts_tiles,
                                                        tblocks):
                            for (b0, b1) in blocks:
                                nc.tensor.matmul(
                                    pt[:, b0:b1], xop[kt][:, ms],
                                    wop[kt][:, c0 + b0:c0 + b1],
                                    start=first, stop=last)
            else:
                # T-outer: finish each PSUM tile early so its scans release it
                # while the next tile still matmuls.
                for (c0, c1), pt, blocks in zip(tspec_m, pts_tiles, tblocks):
                    for kt in range(4):
                        for ti, (xop, wop) in enumerate(terms):
                            first = (kt == 0 and ti == 0)
                            last = (kt == 3 and ti == 2)
                            for (b0, b1) in blocks:
                                nc.tensor.matmul(
                                    pt[:, b0:b1], xop[kt][:, ms],
                                    wop[kt][:, c0 + b0:c0 + b1],
                                    start=first, stop=last)

            def lg(col0, col1):
                """AP view of logits columns [col0, col1) across T tiles."""
                for (c0, c1), pt in zip(tspec_m, pts_tiles):
                    if col0 >= c0 and col1 <= c1:
                        return pt[:, col0 - c0:col1 - c0]
                raise AssertionError((col0, col1))

            # params to SBUF (strided copy PSUM->SBUF)
            a_sb = tiny.tile([128, NSLOTS], f32, tag="a_sb")
            c_sb = tiny.tile([128, NSLOTS], f32, tag="c_sb")
            pview = lg(PARAM_OFF, NOUT).rearrange("p (k two) -> p k two", two=2)
            nc.scalar.copy(out=a_sb[:], in_=pview[:, :, 0:1].squeeze(2))
            nc.scalar.copy(out=c_sb[:], in_=pview[:, :, 1:2].squeeze(2))

            # ================= stage C: argmax ==========================
            rm = work.tile([128, NPTS * PRED], f32, tag="rm")
            pts = tiny.tile([128, NPTS], f32, tag="pts")
            gorder = list(range(NPTS))
            for g in gorder:
                gs = slice(g * PRED, (g + 1) * PRED)
                nc.vector.tensor_tensor_scan(
                    out=rm[:, gs], data0=lg(g * PRED, (g + 1) * PRED),
                    data1=zeros336[:], initial=-3.0e38,
                    op0=Alu.max, op1=Alu.bypass)
            cnt_scr = work.tile([128, PRED], bf16, tag="cntscr")
            for g in gorder:
                gs = slice(g * PRED, (g + 1) * PRED)
                # t* = sum_t sign(m - rm_t)  (1 while rm<m, 0 from argmax on)
                nc.scalar.activation(
                    out=cnt_scr[:], in_=rm[:, gs],
                    func=mybir.ActivationFunctionType.Sign,
                    bias=rm[:, g * PRED + 335:g * PRED + 336], scale=-1.0,
                    accum_out=pts[:, g:g + 1])

            # ================= stage D: combine =========================
            p_ap = pts[:]
            p_i = p_ap.unsqueeze(2).broadcast_to([128, NPTS, NPTS])
            p_j = p_ap.unsqueeze(1).broadcast_to([128, NPTS, NPTS])
            LT = tiny.tile([128, 64], f32, tag="LT")
            LE = tiny.tile([128, 64], f32, tag="LE")
            nc.vector.tensor_tensor(
                out=LT[:].rearrange("p (i j) -> p i j", j=8), in0=p_i, in1=p_j,
                op=Alu.is_gt)   # LT[i,j] = p_j < p_i
            nc.vector.tensor_tensor(
                out=LE[:].rearrange("p (i j) -> p i j", j=8), in0=p_i, in1=p_j,
                op=Alu.is_ge)   # LE[i,j] = p_j <= p_i
            cnt_lt = tiny.tile([128, NPTS], f32, tag="cnt_lt")
            cnt_le = tiny.tile([128, NPTS], f32, tag="cnt_le")
            nc.vector.tensor_reduce(
                out=cnt_lt[:], in_=LT[:].rearrange("p (i j) -> p i j", j=8),
                axis=mybir.AxisListType.X, op=Alu.add)
            nc.vector.tensor_reduce(
                out=cnt_le[:], in_=LE[:].rearrange("p (i j) -> p i j", j=8),
                axis=mybir.AxisListType.X, op=Alu.add)
            EQ = tiny.tile([128, 64], f32, tag="EQ")
            nc.vector.tensor_tensor(out=EQ[:], in0=LE[:], in1=LT[:],
                                    op=Alu.subtract)
            EQt = tiny.tile([128, 64], f32, tag="EQt")
            nc.vector.tensor_tensor(out=EQt[:], in0=EQ[:], in1=tri[:],
                                    op=Alu.mult)
            Ecnt = tiny.tile([128, NPTS], f32, tag="Ecnt")
            nc.vector.tensor_reduce(
                out=Ecnt[:], in_=EQt[:].rearrange("p (i j) -> p i j", j=8),
                axis=mybir.AxisListType.X, op=Alu.add)
            isf = tiny.tile([128, NPTS], f32, tag="isf")
            nc.vector.tensor_scalar(out=isf[:], in0=Ecnt[:], scalar1=0.0,
                                    scalar2=None, op0=Alu.is_equal)

            # one-hot lookups a[cnt], c[cnt]
            i9 = iota9[:].unsqueeze(1).broadcast_to([128, NPTS, NSLOTS])
            a_b = a_sb[:].unsqueeze(1).broadcast_to([128, NPTS, NSLOTS])
            c_b = c_sb[:].unsqueeze(1).broadcast_to([128, NPTS, NSLOTS])

            def onehot_lookup(cnt, tag):
                eq = tiny.tile([128, NPTS * NSLOTS], f32, tag=f"eq{tag}", name=f"eq{tag}")
                eqv = eq[:].rearrange("p (i k) -> p i k", k=NSLOTS)
                nc.vector.tensor_tensor(
                    out=eqv, in0=cnt[:].unsqueeze(2).broadcast_to(
                        [128, NPTS, NSLOTS]), in1=i9, op=Alu.is_equal)
                outs = []
                for nm, tbl in (("a", a_b), ("c", c_b)):
                    prod = tiny.tile([128, NPTS * NSLOTS], f32,
                                     tag=f"pr{nm}{tag}", name=f"pr{nm}{tag}")
                    pv = prod[:].rearrange("p (i k) -> p i k", k=NSLOTS)
                    nc.vector.tensor_tensor(out=pv, in0=eqv, in1=tbl,
                                            op=Alu.mult)
                    red = tiny.tile([128, NPTS], f32, tag=f"rd{nm}{tag}", name=f"rd{nm}{tag}")
                    nc.vector.tensor_reduce(out=red[:], in_=pv,
                                            axis=mybir.AxisListType.X,
                                            op=Alu.add)
                    outs.append(red)
                return outs

            a_le, c_le = onehot_lookup(cnt_le, "le")
            a_lt, c_lt = onehot_lookup(cnt_lt, "lt")

            da = tiny.tile([128, NPTS], f32, tag="da")
            dc = tiny.tile([128, NPTS], f32, tag="dc")
            nc.vector.tensor_tensor(out=da[:], in0=a_le[:], in1=a_lt[:],
                                    op=Alu.subtract)
            nc.vector.tensor_tensor(out=dc[:], in0=c_le[:], in1=c_lt[:],
                                    op=Alu.subtract)
            # j2 = da*(p-1) + dc
            j2a = tiny.tile([128, NPTS], f32, tag="j2a")
            nc.vector.scalar_tensor_tensor(
                out=j2a[:], in0=pts[:], scalar=1.0, in1=da[:],
                op0=Alu.subtract, op1=Alu.mult)
            j2v = tiny.tile([128, NPTS], f32, tag="j2v")
            nc.vector.tensor_tensor(out=j2v[:], in0=j2a[:], in1=dc[:],
                                    op=Alu.add)
            # scatter index: is_first ? p : -1
            p1f = tiny.tile([128, NPTS], f32, tag="p1f")
            nc.vector.scalar_tensor_tensor(
                out=p1f[:], in0=pts[:], scalar=1.0, in1=isf[:],
                op0=Alu.add, op1=Alu.mult)
            sidxf = tiny.tile([128, NPTS], f32, tag="sidxf")
            nc.vector.tensor_scalar(out=sidxf[:], in0=p1f[:], scalar1=1.0,
                                    scalar2=None, op0=Alu.subtract)
            sidx = tiny.tile([128, NPTS], i16, tag="sidx")
            nc.vector.tensor_copy(out=sidx[:], in_=sidxf[:])

            # bf16 hi/lo split of scatter payloads
            def bsplit(src, tag):
                hi = tiny.tile([128, NPTS], bf16, tag=f"{tag}h", name=f"{tag}h")
                nc.vector.tensor_copy(out=hi[:], in_=src[:])
                lo = tiny.tile([128, NPTS], bf16, tag=f"{tag}l", name=f"{tag}l")
                nc.vector.tensor_tensor(out=lo[:], in0=src[:], in1=hi[:],
                                        op=Alu.subtract)
                return hi, lo

            j1h, j1l = bsplit(da, "j1")
            j2h, j2l = bsplit(j2v, "j2")

            scat = []
            for nm, payload in (("s1h", j1h), ("s1l", j1l),
                                ("s2h", j2h), ("s2l", j2l)):
                s = work.tile([128, PRED], bf16, tag=nm, name=nm)
                nc.gpsimd.local_scatter(
                    out_ap=s[:], data_ap=payload[:], idxs_ap=sidx[:],
                    channels=128, num_elems=PRED, num_idxs=NPTS)
                scat.append(s)

            u = work.tile([128, PRED], f32, tag="u")
            nc.vector.tensor_tensor_scan(
                out=u[:], data0=scat[0][:], data1=scat[1][:],
                initial=a_sb[:, 0:1], op0=Alu.add, op1=Alu.add)
            j2s = work.tile([128, PRED], f32, tag="j2s")
            nc.vector.tensor_tensor(out=j2s[:], in0=scat[2][:],
                                    in1=scat[3][:], op=Alu.add)
            init2 = tiny.tile([128, 1], f32, tag="init2")
            nc.vector.scalar_tensor_tensor(
                out=init2[:], in0=c_sb[:, 0:1], scalar=a_sb[:, 0:1],
                in1=lvs[:, m:m + 1], op0=Alu.subtract, op1=Alu.add)
            res = work.tile([128, PRED], f32, tag="res")
            nc.vector.tensor_tensor_scan(
                out=res[:], data0=u[:], data1=j2s[:],
                initial=init2[:], op0=Alu.add, op1=Alu.add)
            nc.sync.dma_start(out_d[ms, :], res[:])

    nc.compile()
    return nc


def _host_prep(inputs):
    x = np.ascontiguousarray(inputs["x"], dtype=np.float32)
    W_pts = np.asarray(inputs["W_pts"], dtype=np.float32)
    b_pts = np.asarray(inputs["b_pts"], dtype=np.float32)
    W_comp = np.asarray(inputs["W_comp"], dtype=np.float32)
    b_comp = np.asarray(inputs["b_comp"], dtype=np.float32)

    Wall = np.concatenate([W_pts, W_comp], axis=0)  # (2706, 512)
    WT = np.ascontiguousarray(Wall.T)               # (512, 2706)
    # Row 511 multiplies xc[:,511] == 0, so it is a free slot: plant the
    # bias there (device side plants a ones-row in xh[3] partition 127).
    ball = np.concatenate([b_pts, b_comp]).astype(np.float32)  # (2706,)
    WT[511, :] = ball
    wh = WT.astype(BF)
    wl = (WT - wh.astype(np.float32)).astype(BF)
    # wl row 511 must hold bias-lo vs wh row 511 (xl row is zeroed, and
    # only xh@wh + xh@wl see the ones-row): already exact by construction.
    ident = np.eye(128, dtype=np.float32)
    iota9 = np.tile(np.arange(NSLOTS, dtype=np.float32), (128, 1))
    tri = np.tile(
        (np.arange(8)[None, :] < np.arange(8)[:, None]).astype(np.float32)
        .reshape(1, 64), (128, 1))
    tri = np.ascontiguousarray(tri)

    shared = dict(wh=wh, wl=wl, ident=ident, iota9=iota9, tri=tri,
                  ones_r=None, zeros_r=None)  # filled per-rows in kernel()
    return x, shared


def kernel(**inputs):
    from concourse.bass_utils import run_bass_kernel_spmd

    x, shared = _host_prep(inputs)
    B = x.shape[0]
    rows = B // N_CORES

    key = rows
    if key not in _CACHE:
        _CACHE[key] = _build(rows)
    nc = _CACHE[key]

    shared["ones_r"] = np.ones((1, rows), dtype=BF)
    shared["zeros_r"] = np.zeros((1, rows), dtype=BF)
    in_maps = []
    for c in range(N_CORES):
        m = {"x": x[c * rows:(c + 1) * rows]}
        m.update(shared)
        in_maps.append(m)

    trace = bool(int(os.environ.get("KERNEL_TRACE", "0")))
    r = run_bass_kernel_spmd(nc, in_maps, list(range(N_CORES)), trace=trace)
    kernel.last_exec_time_ns = r.exec_time_ns
    kernel.last_results = r

    out = np.concatenate([r.results[c]["out"] for c in range(N_CORES)], axis=0)
    return out.astype(np.float32)


kernel.last_exec_time_ns = None
kernel.last_results = None


# revision 13
# speedup vs baseline: 1.2912x; 1.0296x over previous
"""Trainium2 Bass kernel for nn_PointEstimator (segment_reduce).

Computation (per batch row b):
    xc = x - x[:, -1:]
    logits = xc @ W_pts.T + b_pts            # (B, 8*336)
    pts    = sort(argmax over each 336-group) # (B, 8)
    params = (xc @ W_comp.T + b_comp)        # (B, 9, 2) slope/intercept
    out[t] = a[s(t)] * t + c[s(t)] + x[:,-1],  s(t) = #(pts <= t)

Device algorithm highlights:
  * batch rows on PSUM partitions; seq (512) is the contraction dim.
  * matmul in a bf16 3-term split (xh@wh + xh@wl + xl@wh) for fp32-level
    logit accuracy (needed: min top-2 logit gap is ~6e-6); bias folded in
    as a K=2 all-ones matmul that opens each PSUM accumulation group.
  * x is transposed on-device via PE transpose; xc=x-lastcol on DVE.
  * argmax without sort: running-max prefix scan (tensor_tensor_scan max)
    then t* = sum_t [rm_t < rm_335] via scalar_tensor_tensor accum (exact
    first-index tie-break).
  * piecewise combine without sort or gather: for each point, its slot
    jump da = a[cnt_le]-a[cnt_lt] (one-hot lookups via broadcast compare),
    deduplicated to the first point of each tied value, scattered with
    gpsimd local_scatter, then out = double prefix-sum:
        u = a0 + cumsum(j1);  res = (c0+lv-a0) + cumsum(u + j2),
        j1[p] = da, j2[p] = da*(p-1)+dc.
"""

import os
import numpy as np
import ml_dtypes

N_CORES = 8
B_FULL = 8192
SEQ = 512
PRED = 336
NPTS = 8
NSLOTS = 9
NOUT = NPTS * PRED + NSLOTS * 2  # 2688 + 18 = 2706
PARAM_OFF = NPTS * PRED          # 2688

BF = ml_dtypes.bfloat16

_CACHE = {}


def _build(rows):
    """Build + compile the per-core Bass program for `rows` batch rows."""
    import concourse.bass as bass
    import concourse.tile as tile
    from concourse import bacc, mybir
    from concourse import library_config
    from contextlib import ExitStack

    f32 = mybir.dt.float32
    bf16 = mybir.dt.bfloat16
    i16 = mybir.dt.int16
    Alu = mybir.AluOpType

    MT = rows // 128
    assert rows % 128 == 0

    nc = bacc.Bacc("TRN2", target_bir_lowering=False, debug=False)

    x_d = nc.dram_tensor("x", [rows, SEQ], f32, kind="ExternalInput").ap()
    wh_d = nc.dram_tensor("wh", [SEQ, NOUT], bf16, kind="ExternalInput").ap()
    wl_d = nc.dram_tensor("wl", [SEQ, NOUT], bf16, kind="ExternalInput").ap()
    ident_d = nc.dram_tensor("ident", [128, 128], f32, kind="ExternalInput").ap()
    iota9_d = nc.dram_tensor("iota9", [128, NSLOTS], f32, kind="ExternalInput").ap()
    tri_d = nc.dram_tensor("tri", [128, 64], f32, kind="ExternalInput").ap()
    ones_r_d = nc.dram_tensor("ones_r", [1, rows], bf16, kind="ExternalInput").ap()
    zeros_r_d = nc.dram_tensor("zeros_r", [1, rows], bf16, kind="ExternalInput").ap()
    out_d = nc.dram_tensor("out", [rows, PRED], f32, kind="ExternalOutput").ap()

    # Logits live in 3 PSUM tiles of 3 groups / 3 groups / 2 groups+params.
    TSPEC = [
        (0, 1008),        # groups 0..2
        (1008, 2016),     # groups 3..5
        (2016, 2706),     # groups 6..7 + 18 params
    ]

    with tile.TileContext(nc) as tc, ExitStack() as ctx:
        const = ctx.enter_context(tc.tile_pool(name="const", bufs=1))
        wpool = ctx.enter_context(tc.tile_pool(name="w", bufs=1))
        xpool = ctx.enter_context(tc.tile_pool(name="xin", bufs=3))
        xs = ctx.enter_context(tc.tile_pool(name="xsplit", bufs=1))
        work = ctx.enter_context(tc.tile_pool(name="work", bufs=2))
        tiny = ctx.enter_context(tc.tile_pool(name="tiny", bufs=2))
        psum = ctx.enter_context(tc.tile_pool(name="psum", bufs=3, space="PSUM"))
        psumT = ctx.enter_context(tc.tile_pool(name="psumT", bufs=2, space="PSUM"))

        nc.gpsimd.load_library(library_config.local_scatter)

        # ---- constants needed by stage A first (x DMAs must win BW) ----
        ident = const.tile([128, 128], f32)
        nc.sync.dma_start(ident[:], ident_d)
        zeros336 = const.tile([128, PRED], f32)
        nc.vector.memset(zeros336[:], 0.0)

        # ---- persistent transposed x splits ----
        xh = [xs.tile([128, rows], bf16, tag=f"xh{kt}", name=f"xh{kt}") for kt in range(4)]
        xl = [xs.tile([128, rows], bf16, tag=f"xl{kt}", name=f"xl{kt}") for kt in range(4)]
        lvs = xs.tile([128, MT], f32, tag="lvs")  # last value per mtile
        # bias rows: xh[3] partition 127 = 1.0 (multiplies W row 511 = bias),
        # xl[3] partition 127 = 0. Written once; per-mtile copies skip row 127.
        nc.sync.dma_start(xh[3][127:128, :], ones_r_d)
        nc.sync.dma_start(xl[3][127:128, :], zeros_r_d)

        # ======== stage A for ALL mtiles: load, xc, transpose, split ====
        whs, wls = [None] * 4, [None] * 4

        def dma_w(kt):
            t = wpool.tile([128, NOUT], bf16, tag=f"wh{kt}", name=f"wh{kt}")
            nc.sync.dma_start(t[:], wh_d[kt * 128:(kt + 1) * 128, :])
            whs[kt] = t
            t = wpool.tile([128, NOUT], bf16, tag=f"wl{kt}", name=f"wl{kt}")
            nc.sync.dma_start(t[:], wl_d[kt * 128:(kt + 1) * 128, :])
            wls[kt] = t

        for m in range(MT):
            ms = slice(m * 128, (m + 1) * 128)
            xt = xpool.tile([128, SEQ], f32, tag="xt")
            nc.sync.dma_start(xt[:], x_d[ms, :])
            if m < 4:
                dma_w(m)   # stagger W chunks: kt arrives in need order
            xc = xpool.tile([128, SEQ], f32, tag="xc")
            nc.vector.tensor_scalar(
                out=xc[:], in0=xt[:], scalar1=xt[:, 511:512], scalar2=None,
                op0=Alu.subtract)
            nc.scalar.copy(out=lvs[:, m:m + 1], in_=xt[:, 511:512])

            pst = psumT.tile([128, SEQ], f32, tag="ptrans")
            for kt in range(4):
                ks = slice(kt * 128, (kt + 1) * 128)
                nc.tensor.transpose(pst[:, ks], xc[:, ks], ident[:])
            for kt in range(4):
                ks = slice(kt * 128, (kt + 1) * 128)
                np_ = 127 if kt == 3 else 128
                pp = slice(0, np_)
                nc.scalar.copy(out=xh[kt][pp, ms], in_=pst[pp, ks])
                nc.vector.tensor_tensor(
                    out=xl[kt][pp, ms], in0=pst[pp, ks], in1=xh[kt][pp, ms],
                    op=Alu.subtract)

        for kt in range(4):
            if whs[kt] is None:   # rows < 512: finish W DMAs here
                dma_w(kt)
        iota9 = const.tile([128, NSLOTS], f32)
        nc.sync.dma_start(iota9[:], iota9_d)
        tri = const.tile([128, 64], f32)
        nc.sync.dma_start(tri[:], tri_d)

        for m in range(MT):
            ms = slice(m * 128, (m + 1) * 128)
            # ================= stage B: matmuls =========================
            tspec_m = TSPEC
            pts_tiles = []
            tblocks = []
            for (c0, c1) in tspec_m:
                pt = psum.tile([128, c1 - c0], f32, tag="lg", name=f"lg{c0}", padded_shape=[128, 1024])
                pts_tiles.append(pt)
                w = c1 - c0
                blocks = [(0, min(512, w))]
                if w > 512:
                    blocks.append((512, w))
                tblocks.append(blocks)
            terms = ([(xh, whs), (xl, whs), (xh, wls)] if m == 0
                     else [(xh, whs), (xh, wls), (xl, whs)])
            if m == 0:
                # kt-outer: first matmuls need only the kt=0 weight tiles,
                # which stream in while these run (shorter cold preamble).
                for kt in range(4):
                    for ti, (xop, wop) in enumerate(terms):
                        first = (kt == 0 and ti == 0)
                        last = (kt == 3 and ti == 2)
                        for (c0, c1), pt, blocks in zip(tspec_m, pts_tiles,
                                                        tblocks):
                            for (b0, b1) in blocks:
                                nc.tensor.matmul(
                                    pt[:, b0:b1], xop[kt][:, ms],
                                    wop[kt][:, c0 + b0:c0 + b1],
                                    start=first, stop=last)
            else:
                # T-outer: finish each PSUM tile early so its scans release it
                # while the next tile still matmuls.
                for (c0, c1), pt, blocks in zip(tspec_m, pts_tiles, tblocks):
                    for kt in range(4):
                        for ti, (xop, wop) in enumerate(terms):
                            first = (kt == 0 and ti == 0)
                            last = (kt == 3 and ti == 2)
                            for (b0, b1) in blocks:
                                nc.tensor.matmul(
                                    pt[:, b0:b1], xop[kt][:, ms],
                                    wop[kt][:, c0 + b0:c0 + b1],
                                    start=first, stop=last)

            def lg(col0, col1):
                """AP view of logits columns [col0, col1) across T tiles."""
                for (c0, c1), pt in zip(tspec_m, pts_tiles):
                    if col0 >= c0 and col1 <= c1:
                        return pt[:, col0 - c0:col1 - c0]
                raise AssertionError((col0, col1))

            # params to SBUF (strided copy PSUM->SBUF)
            a_sb = tiny.tile([128, NSLOTS], f32, tag="a_sb")
            c_sb = tiny.tile([128, NSLOTS], f32, tag="c_sb")
            pview = lg(PARAM_OFF, NOUT).rearrange("p (k two) -> p k two", two=2)
            nc.scalar.copy(out=a_sb[:], in_=pview[:, :, 0:1].squeeze(2))
            nc.scalar.copy(out=c_sb[:], in_=pview[:, :, 1:2].squeeze(2))

            # ================= stage C: argmax ==========================
            rm = work.tile([128, NPTS * PRED], f32, tag="rm")
            pts = tiny.tile([128, NPTS], f32, tag="pts")
            gorder = list(range(NPTS))
            for g in gorder:
                gs = slice(g * PRED, (g + 1) * PRED)
                nc.vector.tensor_tensor_scan(
                    out=rm[:, gs], data0=lg(g * PRED, (g + 1) * PRED),
                    data1=zeros336[:], initial=-3.0e38,
                    op0=Alu.max, op1=Alu.bypass)
            cnt_scr = work.tile([128, PRED], bf16, tag="cntscr")
            for g in gorder:
                gs = slice(g * PRED, (g + 1) * PRED)
                # t* = sum_t sign(m - rm_t)  (1 while rm<m, 0 from argmax on)
                nc.scalar.activation(
                    out=cnt_scr[:], in_=rm[:, gs],
                    func=mybir.ActivationFunctionType.Sign,
                    bias=rm[:, g * PRED + 335:g * PRED + 336], scale=-1.0,
                    accum_out=pts[:, g:g + 1])

            # ================= stage D: combine =========================
            p_ap = pts[:]
            p_i = p_ap.unsqueeze(2).broadcast_to([128, NPTS, NPTS])
            p_j = p_ap.unsqueeze(1).broadcast_to([128, NPTS, NPTS])
            LT = tiny.tile([128, 64], f32, tag="LT")
            LE = tiny.tile([128, 64], f32, tag="LE")
            nc.vector.tensor_tensor(
                out=LT[:].rearrange("p (i j) -> p i j", j=8), in0=p_i, in1=p_j,
                op=Alu.is_gt)   # LT[i,j] = p_j < p_i
            nc.vector.tensor_tensor(
                out=LE[:].rearrange("p (i j) -> p i j", j=8), in0=p_i, in1=p_j,
                op=Alu.is_ge)   # LE[i,j] = p_j <= p_i
            cnt_lt = tiny.tile([128, NPTS], f32, tag="cnt_lt")
            cnt_le = tiny.tile([128, NPTS], f32, tag="cnt_le")
            nc.vector.tensor_reduce(
                out=cnt_lt[:], in_=LT[:].rearrange("p (i j) -> p i j", j=8),
                axis=mybir.AxisListType.X, op=Alu.add)
            nc.vector.tensor_reduce(
                out=cnt_le[:], in_=LE[:].rearrange("p (i j) -> p i j", j=8),
                axis=mybir.AxisListType.X, op=Alu.add)
            EQ = tiny.tile([128, 64], f32, tag="EQ")
            nc.vector.tensor_tensor(out=EQ[:], in0=LE[:], in1=LT[:],
                                    op=Alu.subtract)
            EQt = tiny.tile([128, 64], f32, tag="EQt")
            nc.vector.tensor_tensor(out=EQt[:], in0=EQ[:], in1=tri[:],
                                    op=Alu.mult)
            Ecnt = tiny.tile([128, NPTS], f32, tag="Ecnt")
            nc.vector.tensor_reduce(
                out=Ecnt[:], in_=EQt[:].rearrange("p (i j) -> p i j", j=8),
                axis=mybir.AxisListType.X, op=Alu.add)
            isf = tiny.tile([128, NPTS], f32, tag="isf")
            nc.vector.tensor_scalar(out=isf[:], in0=Ecnt[:], scalar1=0.0,
                                    scalar2=None, op0=Alu.is_equal)

            # one-hot lookups a[cnt], c[cnt]
            i9 = iota9[:].unsqueeze(1).broadcast_to([128, NPTS, NSLOTS])
            a_b = a_sb[:].unsqueeze(1).broadcast_to([128, NPTS, NSLOTS])
            c_b = c_sb[:].unsqueeze(1).broadcast_to([128, NPTS, NSLOTS])

            def onehot_lookup(cnt, tag):
                eq = tiny.tile([128, NPTS * NSLOTS], f32, tag=f"eq{tag}", name=f"eq{tag}")
                eqv = eq[:].rearrange("p (i k) -> p i k", k=NSLOTS)
                nc.vector.tensor_tensor(
                    out=eqv, in0=cnt[:].unsqueeze(2).broadcast_to(
                        [128, NPTS, NSLOTS]), in1=i9, op=Alu.is_equal)
                outs = []
                for nm, tbl in (("a", a_b), ("c", c_b)):
                    prod = tiny.tile([128, NPTS * NSLOTS], f32,
                                     tag=f"pr{nm}{tag}", name=f"pr{nm}{tag}")
                    pv = prod[:].rearrange("p (i k) -> p i k", k=NSLOTS)
                    nc.vector.tensor_tensor(out=pv, in0=eqv, in1=tbl,
                                            op=Alu.mult)
                    red = tiny.tile([128, NPTS], f32, tag=f"rd{nm}{tag}", name=f"rd{nm}{tag}")
                    nc.vector.tensor_reduce(out=red[:], in_=pv,
                                            axis=mybir.AxisListType.X,
                                            op=Alu.add)
                    outs.append(red)
                return outs

            a_le, c_le = onehot_lookup(cnt_le, "le")
            a_lt, c_lt = onehot_lookup(cnt_lt, "lt")

            da = tiny.tile([128, NPTS], f32, tag="da")
            dc = tiny.tile([128, NPTS], f32, tag="dc")
            nc.vector.tensor_tensor(out=da[:], in0=a_le[:], in1=a_lt[:],
                                    op=Alu.subtract)
            nc.vector.tensor_tensor(out=dc[:], in0=c_le[:], in1=c_lt[:],
                                    op=Alu.subtract)
            # j2 = da*(p-1) + dc
            j2a = tiny.tile([128, NPTS], f32, tag="j2a")
            nc.vector.scalar_tensor_tensor(
                out=j2a[:], in0=pts[:], scalar=1.0, in1=da[:],
                op0=Alu.subtract, op1=Alu.mult)
            j2v = tiny.tile([128, NPTS], f32, tag="j2v")
            nc.vector.tensor_tensor(out=j2v[:], in0=j2a[:], in1=dc[:],
                                    op=Alu.add)
            # scatter index: is_first ? p : -1
            p1f = tiny.tile([128, NPTS], f32, tag="p1f")
            nc.vector.scalar_tensor_tensor(
                out=p1f[:], in0=pts[:], scalar=1.0, in1=isf[:],
                op0=Alu.add, op1=Alu.mult)
            sidxf = tiny.tile([128, NPTS], f32, tag="sidxf")
            nc.vector.tensor_scalar(out=sidxf[:], in0=p1f[:], scalar1=1.0,
                                    scalar2=None, op0=Alu.subtract)
            sidx = tiny.tile([128, NPTS], i16, tag="sidx")
            nc.vector.tensor_copy(out=sidx[:], in_=sidxf[:])

            # bf16 hi/lo split of scatter payloads
            def bsplit(src, tag):
                hi = tiny.tile([128, NPTS], bf16, tag=f"{tag}h", name=f"{tag}h")
                nc.vector.tensor_copy(out=hi[:], in_=src[:])
                lo = tiny.tile([128, NPTS], bf16, tag=f"{tag}l", name=f"{tag}l")
                nc.vector.tensor_tensor(out=lo[:], in0=src[:], in1=hi[:],
                                        op=Alu.subtract)
                return hi, lo

            j1h, j1l = bsplit(da, "j1")
            j2h, j2l = bsplit(j2v, "j2")

            scat = []
            for nm, payload in (("s1h", j1h), ("s1l", j1l),
                                ("s2h", j2h), ("s2l", j2l)):
                s = work.tile([128, PRED], bf16, tag=nm, name=nm)
                nc.gpsimd.local_scatter(
                    out_ap=s[:], data_ap=payload[:], idxs_ap=sidx[:],
                    channels=128, num_elems=PRED, num_idxs=NPTS)
                scat.append(s)

            u = work.tile([128, PRED], f32, tag="u")
            nc.vector.tensor_tensor_scan(
                out=u[:], data0=scat[0][:], data1=scat[1][:],
                initial=a_sb[:, 0:1], op0=Alu.add, op1=Alu.add)
            j2s = work.tile([128, PRED], f32, tag="j2s")
            nc.vector.tensor_tensor(out=j2s[:], in0=scat[2][:],
                                    in1=scat[3][:], op=Alu.add)
            init2 = tiny.tile([128, 1], f32, tag="init2")
            nc.vector.scalar_tensor_tensor(
                out=init2[:], in0=c_sb[:, 0:1], scalar=a_sb[:, 0:1],
                in1=lvs[:, m:m + 1], op0=Alu.subtract, op1=Alu.add)
            res = work.tile([128, PRED], f32, tag="res")
            nc.vector.tensor_tensor_scan(
                out=res[:], data0=u[:], data1=j2s[:],
                initial=init2[:], op0=Alu.add, op1=Alu.add)
            nc.sync.dma_start(out_d[ms, :], res[:])

    nc.compile()
    return nc


def _host_prep(inputs):
    x = np.ascontiguousarray(inputs["x"], dtype=np.float32)
    W_pts = np.asarray(inputs["W_pts"], dtype=np.float32)
    b_pts = np.asarray(inputs["b_pts"], dtype=np.float32)
    W_comp = np.asarray(inputs["W_comp"], dtype=np.float32)
    b_comp = np.asarray(inputs["b_comp"], dtype=np.float32)

    Wall = np.concatenate([W_pts, W_comp], axis=0)  # (2706, 512)
    WT = np.ascontiguousarray(Wall.T)               # (512, 2706)
    # Row 511 multiplies xc[:,511] == 0, so it is a free slot: plant the
    # bias there (device side plants a ones-row in xh[3] partition 127).
    ball = np.concatenate([b_pts, b_comp]).astype(np.float32)  # (2706,)
    WT[511, :] = ball
    wh = WT.astype(BF)
    wl = (WT - wh.astype(np.float32)).astype(BF)
    # wl row 511 must hold bias-lo vs wh row 511 (xl row is zeroed, and
    # only xh@wh + xh@wl see the ones-row): already exact by construction.
    ident = np.eye(128, dtype=np.float32)
    iota9 = np.tile(np.arange(NSLOTS, dtype=np.float32), (128, 1))
    tri = np.tile(
        (np.arange(8)[None, :] < np.arange(8)[:, None]).astype(np.float32)
        .reshape(1, 64), (128, 1))
    tri = np.ascontiguousarray(tri)

    shared = dict(wh=wh, wl=wl, ident=ident, iota9=iota9, tri=tri,
                  ones_r=None, zeros_r=None)  # filled per-rows in kernel()
    return x, shared


def kernel(**inputs):
    from concourse.bass_utils import run_bass_kernel_spmd

    x, shared = _host_prep(inputs)
    B = x.shape[0]
    rows = B // N_CORES

    key = rows
    if key not in _CACHE:
        _CACHE[key] = _build(rows)
    nc = _CACHE[key]

    shared["ones_r"] = np.ones((1, rows), dtype=BF)
    shared["zeros_r"] = np.zeros((1, rows), dtype=BF)
    in_maps = []
    for c in range(N_CORES):
        m = {"x": x[c * rows:(c + 1) * rows]}
        m.update(shared)
        in_maps.append(m)

    trace = bool(int(os.environ.get("KERNEL_TRACE", "0")))
    r = run_bass_kernel_spmd(nc, in_maps, list(range(N_CORES)), trace=trace)
    kernel.last_exec_time_ns = r.exec_time_ns
    kernel.last_results = r

    out = np.concatenate([r.results[c]["out"] for c in range(N_CORES)], axis=0)
    return out.astype(np.float32)


kernel.last_exec_time_ns = None
kernel.last_results = None
